# revision 11
# baseline (speedup 1.0000x reference)
"""Trainium2 Bass kernel for nn_LocalPointDecoder (sparse_attention).

Algorithm (per query point n):
  c[n]  = softmax_m(-|q_n - pp_m|^2 / VAR) @ fea          (Gaussian point attention)
  out[n] = MLP(c[n], q_n)                                  (5-block ResNet MLP, H=256)

Sharding: query points N=16384 split across 8 cores (2048 each); every core
holds the full pp/fea context and all MLP weights.

Device layout is fully transposed (features on partitions, queries on the
free axis):
  - d2^T [m, n] from ONE K=5 matmul of host-packed quadratic-form operands:
      ppack = [-2*pp_x, -2*pp_y, -2*pp_z, |pp|^2, 1]   (lhsT)
      qpack = [q_x, q_y, q_z, 1, |q|^2]                (rhs)
  - W^T = Exp(-d2/VAR) on ACT directly from PSUM (the reference's +EPS inside
    the square perturbs the softmax by <1e-4 relative — validated numerically).
  - c^T [C, n] and the softmax denominator accumulate over 32 m-tiles in PSUM
    (lhsT = fea tile / ones column).
  - MLP runs with H on partitions; weights host-pretransposed, layer biases
    folded into the Relu activations as cumulative per-partition bias vectors,
    the residual stream accumulates directly in PSUM (start=False matmuls).

Hardware constraint honored throughout: a Matmult carries at most ONE sem
wait (single LDWEIGHTS wait slot), so ACT is kept the last reader of every
PSUM bank and DMA waits are absorbed by observer instructions.
"""

import numpy as np

import concourse.bass as bass
import concourse.mybir as mybir
from concourse import bacc
import concourse.tile as tile
from concourse.bass_utils import run_bass_kernel_spmd

F32 = mybir.dt.float32
AF = mybir.ActivationFunctionType

N_CORES = 8
N, M, D, C, H = 16384, 4096, 3, 128, 256
NB = 5
NC_PER = N // N_CORES          # 2048 queries per core
CHUNK = 512                    # free-dim tile (one fp32 PSUM bank)
N_CHUNKS = NC_PER // CHUNK     # 4
MT = M // 128                  # 32 context tiles
INV_VAR = 100.0                # 1 / (0.1**2)
MLP_GROUP = 2                  # sub-chunks resident in PSUM per MLP pass

# packed weight-table column offsets (one DMA for all small constants)
OFF_FCC = 0                      # [128, NB*H]   fc_c_W[i].T h-tiles
OFF_B0 = OFF_FCC + NB * H        # [128, NB*2*H] blk0_W[i].T (kt, ht)
OFF_B1 = OFF_B0 + NB * 2 * H
OFF_FCP = OFF_B1 + NB * 2 * H    # rows 0-2: fc_p_W.T h-tiles
OFF_OW = OFF_FCP + H             # [128, 2] out_W.T k-tiles
OFF_BIAS = OFF_OW + 2            # [128, 22] bias vecs (ht*11 + v)
OFF_ONEC = OFF_BIAS + 22         # [128, 1] ones column
OFF_ONER = OFF_ONEC + 1          # row 0: ones row (128 cols)
WCOLS = OFF_ONER + 128


def build_bass() -> bass.Bass:
    nc = bacc.Bacc()

    qppp_d = nc.declare_dram_parameter("qppp", [5, NC_PER + M], F32, isOutput=False)
    fea_d = nc.declare_dram_parameter("fea", [M, C], F32, isOutput=False)
    wtab_d = nc.declare_dram_parameter("wtab", [128, WCOLS], F32, isOutput=False)
    out_d = nc.declare_dram_parameter("out", [1, NC_PER], F32, isOutput=True)

    with tile.TileContext(nc) as tc:
        with (
            tc.tile_pool(name="consts", bufs=1) as consts,
            tc.tile_pool(name="obspsum", bufs=1, space="PSUM") as obs_pool,
        ):
            fea_sb = consts.tile([128, MT, C], F32, tag="fea")
            nc.sync.dma_start(
                out=fea_sb, in_=fea_d[:, :].rearrange("(t p) c -> p t c", p=128)
            )
            qppp_sb = consts.tile([5, NC_PER + M], F32, tag="qppp")
            nc.sync.dma_start(out=qppp_sb, in_=qppp_d[:, :])
            wt_sb = consts.tile([128, WCOLS], F32, tag="wtab")
            nc.sync.dma_start(out=wt_sb, in_=wtab_d[:, :])

            qp_sb = qppp_sb[:, 0:NC_PER]
            pp_sb = qppp_sb[:, NC_PER : NC_PER + M]
            fcc_lhsT = lambda i, ht: wt_sb[
                :, OFF_FCC + i * H + ht * 128 : OFF_FCC + i * H + ht * 128 + 128
            ]
            b0_lhsT = lambda i, kt, ht: wt_sb[
                :,
                OFF_B0 + i * 512 + kt * 256 + ht * 128 : OFF_B0
                + i * 512 + kt * 256 + ht * 128 + 128,
            ]
            b1_lhsT = lambda i, kt, ht: wt_sb[
                :,
                OFF_B1 + i * 512 + kt * 256 + ht * 128 : OFF_B1
                + i * 512 + kt * 256 + ht * 128 + 128,
            ]
            fcp_lhsT = lambda ht: wt_sb[0:D, OFF_FCP + ht * 128 : OFF_FCP + ht * 128 + 128]
            ow_lhsT = lambda kt: wt_sb[:, OFF_OW + kt : OFF_OW + kt + 1]
            bias_ap = lambda ht, v: wt_sb[:, OFF_BIAS + ht * 11 + v : OFF_BIAS + ht * 11 + v + 1]
            ones_col = wt_sb[:, OFF_ONEC : OFF_ONEC + 1]
            ones_row = wt_sb[0:1, OFF_ONER : OFF_ONER + 128]

            cn_sb = consts.tile([C, NC_PER], F32, tag="cn")  # normalized c^T

            # observer matmul: absorbs the fea DMA wait into the PE clock so
            # the first c^T matmul only waits on ACT
            obs_ps = obs_pool.tile([1, 1], F32, tag="obs")
            nc.tensor.matmul(
                obs_ps, lhsT=fea_sb[0:1, 0, 0:1], rhs=fea_sb[0:1, 0, 0:1],
                start=True, stop=True,
            )

            # ---------------- attention phase ----------------
            with (
                tc.tile_pool(name="spsum", bufs=3, space="PSUM") as s_pool,
                tc.tile_pool(name="ctpsum", bufs=2, space="PSUM") as c_pool,
                tc.tile_pool(name="dnpsum", bufs=2, space="PSUM") as d_pool,
                tc.tile_pool(name="wsb", bufs=3) as w_pool,
                tc.tile_pool(name="rsb", bufs=2) as r_pool,
            ):
                for ch in range(N_CHUNKS):
                    nsl = slice(ch * CHUNK, (ch + 1) * CHUNK)
                    ct_ps = c_pool.tile([C, CHUNK], F32, tag="ct")
                    dn_ps = d_pool.tile([1, CHUNK], F32, tag="dn")

                    def emit_s(t, nsl=nsl):
                        s_ps = s_pool.tile([128, CHUNK], F32, tag="s")
                        nc.tensor.matmul(
                            s_ps,
                            lhsT=pp_sb[:, t * 128 : (t + 1) * 128],
                            rhs=qp_sb[:, nsl],
                            start=True,
                            stop=True,
                        )
                        return s_ps

                    # software-pipeline the d2 matmuls two tiles ahead so the
                    # PE never waits on the ACT exp of the current tile
                    s_tiles = {0: emit_s(0), 1: emit_s(1)}
                    for t in range(MT):
                        w_sb = w_pool.tile([128, CHUNK], F32, tag="w")
                        nc.scalar.activation(
                            w_sb, s_tiles.pop(t), AF.Exp, scale=-INV_VAR
                        )
                        if t + 2 < MT:
                            s_tiles[t + 2] = emit_s(t + 2)
                        nc.tensor.matmul(
                            ct_ps,
                            lhsT=fea_sb[:, t, :],
                            rhs=w_sb,
                            start=(t == 0),
                            stop=(t == MT - 1),
                        )
                        nc.tensor.matmul(
                            dn_ps,
                            lhsT=ones_col,
                            rhs=w_sb,
                            start=(t == 0),
                            stop=(t == MT - 1),
                        )

                    # normalization: r = 1/denom, broadcast over partitions
                    # via a K=1 ones matmul, then scale c^T.  Every PSUM tile's
                    # last reader stays ACT (single-wait rule).
                    dn_sb = r_pool.tile([1, CHUNK], F32, tag="dnc")
                    nc.scalar.activation(dn_sb, dn_ps, AF.Copy)
                    r_sb = r_pool.tile([1, CHUNK], F32, tag="r")
                    nc.vector.reciprocal(r_sb, dn_sb)
                    rb_ps = s_pool.tile([128, CHUNK], F32, tag="s")
                    nc.tensor.matmul(
                        rb_ps, lhsT=ones_row, rhs=r_sb, start=True, stop=True
                    )
                    rb_sb = w_pool.tile([128, CHUNK], F32, tag="w")
                    nc.scalar.activation(rb_sb, rb_ps, AF.Copy)
                    ct_sb = w_pool.tile([C, CHUNK], F32, tag="w")
                    nc.scalar.activation(ct_sb, ct_ps, AF.Copy)
                    nc.vector.tensor_mul(cn_sb[:, nsl], ct_sb, rb_sb)

            # ---------------- MLP phase ----------------
            # net^T stays resident in PSUM per (ht, sub); blk1/fc_c matmuls
            # accumulate the residual stream in place.  Two sub-chunks at a
            # time: 2 ht x 2 sub net banks + 3 h banks + obs = 8 banks.
            with (
                tc.tile_pool(name="netpsum", bufs=2 * MLP_GROUP, space="PSUM") as n_pool,
                tc.tile_pool(name="hpsum", bufs=3, space="PSUM") as h_pool,
                tc.tile_pool(name="asb", bufs=4) as a_pool,
                tc.tile_pool(name="bsb", bufs=4) as b_pool,
                tc.tile_pool(name="osb", bufs=2) as os_pool,
            ):
                for g0 in range(0, N_CHUNKS, MLP_GROUP):
                    subs = range(g0, min(g0 + MLP_GROUP, N_CHUNKS))
                    net = {}
                    for sub in subs:
                        nsl = slice(sub * CHUNK, (sub + 1) * CHUNK)
                        for ht in range(2):
                            net_ps = n_pool.tile([128, CHUNK], F32, tag="net")
                            nc.tensor.matmul(
                                net_ps, lhsT=fcp_lhsT(ht), rhs=qp_sb[0:D, nsl],
                                start=True, stop=False,
                            )
                            nc.tensor.matmul(
                                net_ps, lhsT=fcc_lhsT(0, ht), rhs=cn_sb[:, nsl],
                                start=False, stop=True,
                            )
                            net[(ht, sub)] = net_ps

                    for i in range(NB):
                        for sub in subs:
                            nsl = slice(sub * CHUNK, (sub + 1) * CHUNK)
                            rx = []
                            for ht in range(2):
                                rx_sb = a_pool.tile([128, CHUNK], F32, tag="rx")
                                nc.scalar.activation(
                                    rx_sb, net[(ht, sub)], AF.Relu,
                                    bias=bias_ap(ht, i),
                                )
                                rx.append(rx_sb)
                            h_tiles = []
                            for ht in range(2):
                                h_ps = h_pool.tile([128, CHUNK], F32, tag="h")
                                nc.tensor.matmul(
                                    h_ps, lhsT=b0_lhsT(i, 0, ht), rhs=rx[0],
                                    start=True, stop=False,
                                )
                                nc.tensor.matmul(
                                    h_ps, lhsT=b0_lhsT(i, 1, ht), rhs=rx[1],
                                    start=False, stop=True,
                                )
                                h_tiles.append(h_ps)
                            rh = []
                            for ht in range(2):
                                rh_sb = b_pool.tile([128, CHUNK], F32, tag="rh")
                                nc.scalar.activation(
                                    rh_sb, h_tiles[ht], AF.Relu,
                                    bias=bias_ap(ht, 6 + i),
                                )
                                rh.append(rh_sb)
                            last = i == NB - 1
                            for ht in range(2):
                                nc.tensor.matmul(
                                    net[(ht, sub)], lhsT=b1_lhsT(i, 0, ht),
                                    rhs=rh[0], start=False, stop=False,
                                    skip_group_check=True,
                                )
                                nc.tensor.matmul(
                                    net[(ht, sub)], lhsT=b1_lhsT(i, 1, ht),
                                    rhs=rh[1], start=False, stop=last,
                                    skip_group_check=True,
                                )
                                if not last:
                                    nc.tensor.matmul(
                                        net[(ht, sub)], lhsT=fcc_lhsT(i + 1, ht),
                                        rhs=cn_sb[:, nsl], start=False, stop=True,
                                        skip_group_check=True,
                                    )

                    # out = out_W @ relu(net + B_y)   (+ out_b added on host)
                    for sub in subs:
                        nsl = slice(sub * CHUNK, (sub + 1) * CHUNK)
                        ry = []
                        for ht in range(2):
                            ry_sb = a_pool.tile([128, CHUNK], F32, tag="rx")
                            nc.scalar.activation(
                                ry_sb, net[(ht, sub)], AF.Relu, bias=bias_ap(ht, 5)
                            )
                            ry.append(ry_sb)
                        o_ps = h_pool.tile([1, CHUNK], F32, tag="h")
                        nc.tensor.matmul(
                            o_ps, lhsT=ow_lhsT(0), rhs=ry[0], start=True, stop=False
                        )
                        nc.tensor.matmul(
                            o_ps, lhsT=ow_lhsT(1), rhs=ry[1], start=False, stop=True
                        )
                        out_sb = os_pool.tile([1, CHUNK], F32, tag="osb")
                        nc.scalar.activation(out_sb, o_ps, AF.Copy)
                        nc.sync.dma_start(out=out_d[:, nsl], in_=out_sb)

    return nc


def host_prep(inputs):
    p = np.asarray(inputs["p"], np.float32)[0]      # [N, 3]
    pp = np.asarray(inputs["pp"], np.float32)[0]    # [M, 3]
    fea = np.ascontiguousarray(np.asarray(inputs["fea"], np.float32)[0])  # [M, C]

    qpack = np.empty((5, N), np.float32)
    qpack[0:3] = p.T
    qpack[3] = 1.0
    qpack[4] = (p * p).sum(1)
    ppack = np.empty((5, M), np.float32)
    ppack[0:3] = -2.0 * pp.T
    ppack[3] = (pp * pp).sum(1)
    ppack[4] = 1.0

    fc_p_W = np.asarray(inputs["fc_p_W"], np.float32)    # [H, 3]
    fc_c_W = np.asarray(inputs["fc_c_W"], np.float32)    # [NB, H, C]
    blk0_W = np.asarray(inputs["blk0_W"], np.float32)    # [NB, H, H]
    blk1_W = np.asarray(inputs["blk1_W"], np.float32)
    out_W = np.asarray(inputs["out_W"], np.float32)      # [1, H]
    fc_p_b = np.asarray(inputs["fc_p_b"], np.float32)
    fc_c_b = np.asarray(inputs["fc_c_b"], np.float32)    # [NB, H]
    blk0_b = np.asarray(inputs["blk0_b"], np.float32)
    blk1_b = np.asarray(inputs["blk1_b"], np.float32)
    out_b = float(np.asarray(inputs["out_b"], np.float32)[0])

    wtab = np.zeros((128, WCOLS), np.float32)
    wtab[:, OFF_FCC : OFF_FCC + NB * H] = fc_c_W.transpose(2, 0, 1).reshape(C, NB * H)
    wtab[:, OFF_B0 : OFF_B0 + NB * 2 * H] = (
        blk0_W.reshape(NB, H, 2, 128).transpose(3, 0, 2, 1).reshape(128, NB * 2 * H)
    )
    wtab[:, OFF_B1 : OFF_B1 + NB * 2 * H] = (
        blk1_W.reshape(NB, H, 2, 128).transpose(3, 0, 2, 1).reshape(128, NB * 2 * H)
    )
    wtab[0:D, OFF_FCP : OFF_FCP + H] = fc_p_W.T
    wtab[:, OFF_OW : OFF_OW + 2] = out_W.reshape(2, 128).T

    # cumulative bias vectors folded into the Relu activations:
    #   vec 0..4  = B_i  (bias of net before block i's first relu)
    #   vec 5     = B_y  (bias of net before the final relu)
    #   vec 6..10 = blk0_b[i]  (bias of h before block i's second relu)
    vecs = np.zeros((11, H), np.float32)
    B = fc_p_b + fc_c_b[0]
    for i in range(NB):
        vecs[i] = B
        vecs[6 + i] = blk0_b[i]
        B = B + blk1_b[i] + (fc_c_b[i + 1] if i + 1 < NB else 0.0)
    vecs[5] = B
    wtab[:, OFF_BIAS : OFF_BIAS + 22] = (
        vecs.reshape(11, 2, 128).transpose(2, 1, 0).reshape(128, 22)
    )
    wtab[:, OFF_ONEC] = 1.0
    wtab[0, OFF_ONER : OFF_ONER + 128] = 1.0

    shared = {"fea": fea, "wtab": wtab}
    in_maps = []
    for c in range(N_CORES):
        m = dict(shared)
        qppp = np.empty((5, NC_PER + M), np.float32)
        qppp[:, 0:NC_PER] = qpack[:, c * NC_PER : (c + 1) * NC_PER]
        qppp[:, NC_PER:] = ppack
        m["qppp"] = qppp
        in_maps.append(m)
    return in_maps, out_b


_NC_CACHE = {}


def kernel(**inputs) -> np.ndarray:
    in_maps, out_b = host_prep(inputs)
    if "nc" not in _NC_CACHE:
        nc = build_bass()
        nc.finalize()
        _NC_CACHE["nc"] = nc
    nc = _NC_CACHE["nc"]
    res = run_bass_kernel_spmd(nc, in_maps, list(range(N_CORES)))
    parts = [res.results[c]["out"] for c in range(N_CORES)]
    out = np.concatenate(parts, axis=1).astype(np.float32) + np.float32(out_b)
    return out


# revision 12
# speedup vs baseline: 1.3284x; 1.3284x over previous
"""Trainium2 Bass kernel for nn_LocalPointDecoder (sparse_attention).

Algorithm (per query point n):
  c[n]  = softmax_m(-|q_n - pp_m|^2 / VAR) @ fea          (Gaussian point attention)
  out[n] = MLP(c[n], q_n)                                  (5-block ResNet MLP, H=256)

Sharding: query points N=16384 split across 8 cores (2048 each); every core
holds the full pp/fea context and all MLP weights.

Device layout is fully transposed (features on partitions, queries on the
free axis):
  - d2^T [m, n] from ONE K=5 matmul of host-packed quadratic-form operands:
      ppack = [-2*pp_x, -2*pp_y, -2*pp_z, |pp|^2, 1]   (lhsT)
      qpack = [q_x, q_y, q_z, 1, |q|^2]                (rhs)
  - W^T = Exp(-d2/VAR) on ACT directly from PSUM (the reference's +EPS inside
    the square perturbs the softmax by <1e-4 relative — validated numerically).
  - c^T [C, n] and the softmax denominator accumulate over 32 m-tiles in PSUM
    (lhsT = fea tile / ones column).
  - MLP runs with H on partitions; weights host-pretransposed, layer biases
    folded into the Relu activations as cumulative per-partition bias vectors,
    the residual stream accumulates directly in PSUM (start=False matmuls).

Hardware constraint honored throughout: a Matmult carries at most ONE sem
wait (single LDWEIGHTS wait slot), so ACT is kept the last reader of every
PSUM bank and DMA waits are absorbed by observer instructions.
"""

import numpy as np

import concourse.bass as bass
import concourse.mybir as mybir
from concourse import bacc
import concourse.tile as tile
from concourse.bass_utils import run_bass_kernel_spmd

F32 = mybir.dt.float32
BF16 = mybir.dt.bfloat16
AF = mybir.ActivationFunctionType

N_CORES = 8
N, M, D, C, H = 16384, 4096, 3, 128, 256
NB = 5
NC_PER = N // N_CORES          # 2048 queries per core
CHUNK = 512                    # free-dim tile (one fp32 PSUM bank)
N_CHUNKS = NC_PER // CHUNK     # 4
MT = M // 128                  # 32 context tiles
INV_VAR = 100.0                # 1 / (0.1**2)
MLP_GROUP = 2                  # sub-chunks resident in PSUM per MLP pass

# packed bf16 weight-table column offsets (one DMA for all matmul weights)
OFF_FCC = 0                      # [128, NB*H]   fc_c_W[i].T h-tiles
OFF_B0 = OFF_FCC + NB * H        # [128, NB*2*H] blk0_W[i].T (kt, ht)
OFF_B1 = OFF_B0 + NB * 2 * H
OFF_OW = OFF_B1 + NB * 2 * H     # [128, 2] out_W.T k-tiles
OFF_ONEC = OFF_OW + 2            # [128, 1] ones column (softmax denominator)
WCOLS = OFF_ONEC + 1
# f32 table: relu biases + fc_p weights + broadcast ones row
OFF_BIAS = 0                     # [128, 22] bias vecs (ht*11 + v)
OFF_FCP = OFF_BIAS + 22          # rows 0-2: fc_p_W.T h-tiles
OFF_ONER = OFF_FCP + H           # row 0: ones row (128 cols)
FCOLS = OFF_ONER + 128


def build_bass() -> bass.Bass:
    nc = bacc.Bacc()

    qppp_d = nc.declare_dram_parameter("qppp", [5, NC_PER + M], F32, isOutput=False)
    fea_d = nc.declare_dram_parameter("fea", [M, C], BF16, isOutput=False)
    wtab_d = nc.declare_dram_parameter("wtab", [128, WCOLS], BF16, isOutput=False)
    ftab_d = nc.declare_dram_parameter("ftab", [128, FCOLS], F32, isOutput=False)
    out_d = nc.declare_dram_parameter("out", [1, NC_PER], F32, isOutput=True)

    with tile.TileContext(nc) as tc:
        with (
            tc.tile_pool(name="consts", bufs=1) as consts,
            tc.tile_pool(name="obspsum", bufs=1, space="PSUM") as obs_pool,
        ):
            fea_sb = consts.tile([128, MT, C], BF16, tag="fea")
            nc.sync.dma_start(
                out=fea_sb, in_=fea_d[:, :].rearrange("(t p) c -> p t c", p=128)
            )
            qppp_sb = consts.tile([5, NC_PER + M], F32, tag="qppp")
            nc.sync.dma_start(out=qppp_sb, in_=qppp_d[:, :])
            wt_sb = consts.tile([128, WCOLS], BF16, tag="wtab")
            nc.sync.dma_start(out=wt_sb, in_=wtab_d[:, :])
            ft_sb = consts.tile([128, FCOLS], F32, tag="ftab")
            nc.sync.dma_start(out=ft_sb, in_=ftab_d[:, :])

            qp_sb = qppp_sb[:, 0:NC_PER]
            pp_sb = qppp_sb[:, NC_PER : NC_PER + M]
            fcc_lhsT = lambda i, ht: wt_sb[
                :, OFF_FCC + i * H + ht * 128 : OFF_FCC + i * H + ht * 128 + 128
            ]
            b0_lhsT = lambda i, kt, ht: wt_sb[
                :,
                OFF_B0 + i * 512 + kt * 256 + ht * 128 : OFF_B0
                + i * 512 + kt * 256 + ht * 128 + 128,
            ]
            b1_lhsT = lambda i, kt, ht: wt_sb[
                :,
                OFF_B1 + i * 512 + kt * 256 + ht * 128 : OFF_B1
                + i * 512 + kt * 256 + ht * 128 + 128,
            ]
            fcp_lhsT = lambda ht: ft_sb[0:D, OFF_FCP + ht * 128 : OFF_FCP + ht * 128 + 128]
            ow_lhsT = lambda kt: wt_sb[:, OFF_OW + kt : OFF_OW + kt + 1]
            bias_ap = lambda ht, v: ft_sb[:, OFF_BIAS + ht * 11 + v : OFF_BIAS + ht * 11 + v + 1]
            ones_col = wt_sb[:, OFF_ONEC : OFF_ONEC + 1]
            ones_row = ft_sb[0:1, OFF_ONER : OFF_ONER + 128]

            cn_sb = consts.tile([C, NC_PER], BF16, tag="cn")  # normalized c^T

            # observer matmul: absorbs the fea DMA wait into the PE clock so
            # the first c^T matmul only waits on ACT
            obs_ps = obs_pool.tile([1, 1], F32, tag="obs")
            nc.tensor.matmul(
                obs_ps, lhsT=fea_sb[0:1, 0, 0:1], rhs=fea_sb[0:1, 0, 0:1],
                start=True, stop=True,
            )

            # ---------------- attention phase ----------------
            with (
                tc.tile_pool(name="spsum", bufs=3, space="PSUM") as s_pool,
                tc.tile_pool(name="ctpsum", bufs=2, space="PSUM") as c_pool,
                tc.tile_pool(name="dnpsum", bufs=2, space="PSUM") as d_pool,
                tc.tile_pool(name="wsb", bufs=3) as w_pool,
                tc.tile_pool(name="rsb", bufs=2) as r_pool,
            ):
                for ch in range(N_CHUNKS):
                    nsl = slice(ch * CHUNK, (ch + 1) * CHUNK)
                    ct_ps = c_pool.tile([C, CHUNK], F32, tag="ct")
                    dn_ps = d_pool.tile([1, CHUNK], F32, tag="dn")

                    def emit_s(t, nsl=nsl):
                        s_ps = s_pool.tile([128, CHUNK], F32, tag="s")
                        nc.tensor.matmul(
                            s_ps,
                            lhsT=pp_sb[:, t * 128 : (t + 1) * 128],
                            rhs=qp_sb[:, nsl],
                            start=True,
                            stop=True,
                        )
                        return s_ps

                    # software-pipeline the d2 matmuls two tiles ahead so the
                    # PE never waits on the ACT exp of the current tile
                    s_tiles = {0: emit_s(0), 1: emit_s(1)}
                    for t in range(MT):
                        w_sb = w_pool.tile([128, CHUNK], BF16, tag="w")
                        nc.scalar.activation(
                            w_sb, s_tiles.pop(t), AF.Exp, scale=-INV_VAR
                        )
                        if t + 2 < MT:
                            s_tiles[t + 2] = emit_s(t + 2)
                        nc.tensor.matmul(
                            ct_ps,
                            lhsT=fea_sb[:, t, :],
                            rhs=w_sb,
                            start=(t == 0),
                            stop=(t == MT - 1),
                        )
                        nc.tensor.matmul(
                            dn_ps,
                            lhsT=ones_col,
                            rhs=w_sb,
                            start=(t == 0),
                            stop=(t == MT - 1),
                        )

                    # normalization: r = 1/denom, broadcast over partitions
                    # via a K=1 ones matmul, then scale c^T.  Every PSUM tile's
                    # last reader stays ACT (single-wait rule).
                    dn_sb = r_pool.tile([1, CHUNK], F32, tag="dnc")
                    nc.scalar.activation(dn_sb, dn_ps, AF.Copy)
                    r_sb = r_pool.tile([1, CHUNK], F32, tag="r")
                    nc.vector.reciprocal(r_sb, dn_sb)
                    rb_ps = s_pool.tile([128, CHUNK], F32, tag="s")
                    nc.tensor.matmul(
                        rb_ps, lhsT=ones_row, rhs=r_sb, start=True, stop=True
                    )
                    rb_sb = r_pool.tile([128, CHUNK], F32, tag="rbc")
                    nc.scalar.activation(rb_sb, rb_ps, AF.Copy)
                    ct_sb = r_pool.tile([C, CHUNK], F32, tag="ctc")
                    nc.scalar.activation(ct_sb, ct_ps, AF.Copy)
                    nc.vector.tensor_mul(cn_sb[:, nsl], ct_sb, rb_sb)

            # ---------------- MLP phase ----------------
            # net^T stays resident in PSUM per (ht, sub); blk1/fc_c matmuls
            # accumulate the residual stream in place.  Two sub-chunks at a
            # time: 2 ht x 2 sub net banks + 3 h banks + obs = 8 banks.
            with (
                tc.tile_pool(name="netpsum", bufs=2 * MLP_GROUP, space="PSUM") as n_pool,
                tc.tile_pool(name="hpsum", bufs=3, space="PSUM") as h_pool,
                tc.tile_pool(name="asb", bufs=4) as a_pool,
                tc.tile_pool(name="bsb", bufs=4) as b_pool,
                tc.tile_pool(name="osb", bufs=2) as os_pool,
            ):
                for g0 in range(0, N_CHUNKS, MLP_GROUP):
                    subs = range(g0, min(g0 + MLP_GROUP, N_CHUNKS))
                    net = {}
                    for sub in subs:
                        nsl = slice(sub * CHUNK, (sub + 1) * CHUNK)
                        for ht in range(2):
                            net_ps = n_pool.tile([128, CHUNK], F32, tag="net")
                            nc.tensor.matmul(
                                net_ps, lhsT=fcp_lhsT(ht), rhs=qp_sb[0:D, nsl],
                                start=True, stop=False,
                            )
                            nc.tensor.matmul(
                                net_ps, lhsT=fcc_lhsT(0, ht), rhs=cn_sb[:, nsl],
                                start=False, stop=True,
                            )
                            net[(ht, sub)] = net_ps

                    for i in range(NB):
                        for sub in subs:
                            nsl = slice(sub * CHUNK, (sub + 1) * CHUNK)
                            rx = []
                            for ht in range(2):
                                rx_sb = a_pool.tile([128, CHUNK], BF16, tag="rx")
                                nc.scalar.activation(
                                    rx_sb, net[(ht, sub)], AF.Relu,
                                    bias=bias_ap(ht, i),
                                )
                                rx.append(rx_sb)
                            h_tiles = []
                            for ht in range(2):
                                h_ps = h_pool.tile([128, CHUNK], F32, tag="h")
                                nc.tensor.matmul(
                                    h_ps, lhsT=b0_lhsT(i, 0, ht), rhs=rx[0],
                                    start=True, stop=False,
                                )
                                nc.tensor.matmul(
                                    h_ps, lhsT=b0_lhsT(i, 1, ht), rhs=rx[1],
                                    start=False, stop=True,
                                )
                                h_tiles.append(h_ps)
                            rh = []
                            for ht in range(2):
                                rh_sb = b_pool.tile([128, CHUNK], BF16, tag="rh")
                                nc.scalar.activation(
                                    rh_sb, h_tiles[ht], AF.Relu,
                                    bias=bias_ap(ht, 6 + i),
                                )
                                rh.append(rh_sb)
                            last = i == NB - 1
                            for ht in range(2):
                                nc.tensor.matmul(
                                    net[(ht, sub)], lhsT=b1_lhsT(i, 0, ht),
                                    rhs=rh[0], start=False, stop=False,
                                    skip_group_check=True,
                                )
                                nc.tensor.matmul(
                                    net[(ht, sub)], lhsT=b1_lhsT(i, 1, ht),
                                    rhs=rh[1], start=False, stop=last,
                                    skip_group_check=True,
                                )
                                if not last:
                                    nc.tensor.matmul(
                                        net[(ht, sub)], lhsT=fcc_lhsT(i + 1, ht),
                                        rhs=cn_sb[:, nsl], start=False, stop=True,
                                        skip_group_check=True,
                                    )

                    # out = out_W @ relu(net + B_y)   (+ out_b added on host)
                    for sub in subs:
                        nsl = slice(sub * CHUNK, (sub + 1) * CHUNK)
                        ry = []
                        for ht in range(2):
                            ry_sb = a_pool.tile([128, CHUNK], BF16, tag="rx")
                            nc.scalar.activation(
                                ry_sb, net[(ht, sub)], AF.Relu, bias=bias_ap(ht, 5)
                            )
                            ry.append(ry_sb)
                        o_ps = h_pool.tile([1, CHUNK], F32, tag="h")
                        nc.tensor.matmul(
                            o_ps, lhsT=ow_lhsT(0), rhs=ry[0], start=True, stop=False
                        )
                        nc.tensor.matmul(
                            o_ps, lhsT=ow_lhsT(1), rhs=ry[1], start=False, stop=True
                        )
                        out_sb = os_pool.tile([1, CHUNK], F32, tag="osb")
                        nc.scalar.activation(out_sb, o_ps, AF.Copy)
                        nc.sync.dma_start(out=out_d[:, nsl], in_=out_sb)

    return nc


def host_prep(inputs):
    p = np.asarray(inputs["p"], np.float32)[0]      # [N, 3]
    pp = np.asarray(inputs["pp"], np.float32)[0]    # [M, 3]
    fea = np.ascontiguousarray(np.asarray(inputs["fea"], np.float32)[0])  # [M, C]

    qpack = np.empty((5, N), np.float32)
    qpack[0:3] = p.T
    qpack[3] = 1.0
    qpack[4] = (p * p).sum(1)
    ppack = np.empty((5, M), np.float32)
    ppack[0:3] = -2.0 * pp.T
    ppack[3] = (pp * pp).sum(1)
    ppack[4] = 1.0

    fc_p_W = np.asarray(inputs["fc_p_W"], np.float32)    # [H, 3]
    fc_c_W = np.asarray(inputs["fc_c_W"], np.float32)    # [NB, H, C]
    blk0_W = np.asarray(inputs["blk0_W"], np.float32)    # [NB, H, H]
    blk1_W = np.asarray(inputs["blk1_W"], np.float32)
    out_W = np.asarray(inputs["out_W"], np.float32)      # [1, H]
    fc_p_b = np.asarray(inputs["fc_p_b"], np.float32)
    fc_c_b = np.asarray(inputs["fc_c_b"], np.float32)    # [NB, H]
    blk0_b = np.asarray(inputs["blk0_b"], np.float32)
    blk1_b = np.asarray(inputs["blk1_b"], np.float32)
    out_b = float(np.asarray(inputs["out_b"], np.float32)[0])

    import ml_dtypes
    wtab = np.zeros((128, WCOLS), ml_dtypes.bfloat16)
    wtab[:, OFF_FCC : OFF_FCC + NB * H] = fc_c_W.transpose(2, 0, 1).reshape(C, NB * H)
    wtab[:, OFF_B0 : OFF_B0 + NB * 2 * H] = (
        blk0_W.reshape(NB, H, 2, 128).transpose(3, 0, 2, 1).reshape(128, NB * 2 * H)
    )
    wtab[:, OFF_B1 : OFF_B1 + NB * 2 * H] = (
        blk1_W.reshape(NB, H, 2, 128).transpose(3, 0, 2, 1).reshape(128, NB * 2 * H)
    )
    wtab[:, OFF_OW : OFF_OW + 2] = out_W.reshape(2, 128).T
    wtab[:, OFF_ONEC] = 1.0
    ftab = np.zeros((128, FCOLS), np.float32)
    ftab[0:D, OFF_FCP : OFF_FCP + H] = fc_p_W.T

    # cumulative bias vectors folded into the Relu activations:
    #   vec 0..4  = B_i  (bias of net before block i's first relu)
    #   vec 5     = B_y  (bias of net before the final relu)
    #   vec 6..10 = blk0_b[i]  (bias of h before block i's second relu)
    vecs = np.zeros((11, H), np.float32)
    B = fc_p_b + fc_c_b[0]
    for i in range(NB):
        vecs[i] = B
        vecs[6 + i] = blk0_b[i]
        B = B + blk1_b[i] + (fc_c_b[i + 1] if i + 1 < NB else 0.0)
    vecs[5] = B
    ftab[:, OFF_BIAS : OFF_BIAS + 22] = (
        vecs.reshape(11, 2, 128).transpose(2, 1, 0).reshape(128, 22)
    )
    ftab[0, OFF_ONER : OFF_ONER + 128] = 1.0

    shared = {"fea": fea.astype(ml_dtypes.bfloat16), "wtab": wtab, "ftab": ftab}
    in_maps = []
    for c in range(N_CORES):
        m = dict(shared)
        qppp = np.empty((5, NC_PER + M), np.float32)
        qppp[:, 0:NC_PER] = qpack[:, c * NC_PER : (c + 1) * NC_PER]
        qppp[:, NC_PER:] = ppack
        m["qppp"] = qppp
        in_maps.append(m)
    return in_maps, out_b


_NC_CACHE = {}


def kernel(**inputs) -> np.ndarray:
    in_maps, out_b = host_prep(inputs)
    if "nc" not in _NC_CACHE:
        nc = build_bass()
        nc.finalize()
        _NC_CACHE["nc"] = nc
    nc = _NC_CACHE["nc"]
    res = run_bass_kernel_spmd(nc, in_maps, list(range(N_CORES)))
    parts = [res.results[c]["out"] for c in range(N_CORES)]
    out = np.concatenate(parts, axis=1).astype(np.float32) + np.float32(out_b)
    return out


# revision 13
# speedup vs baseline: 2.0121x; 1.5147x over previous
"""Trainium2 Bass kernel for nn_LocalPointDecoder (sparse_attention).

Algorithm (per query point n):
  c[n]  = softmax_m(-|q_n - pp_m|^2 / VAR) @ fea          (Gaussian point attention)
  out[n] = MLP(c[n], q_n)                                  (5-block ResNet MLP, H=256)

Sharding: query points N=16384 split across 8 cores (2048 each); every core
holds the full pp/fea context and all MLP weights.

Device layout is fully transposed (features on partitions, queries on the
free axis):
  - d2^T [m, n] from ONE K=5 matmul of host-packed quadratic-form operands:
      ppack = [-2*pp_x, -2*pp_y, -2*pp_z, |pp|^2, 1]   (lhsT)
      qpack = [q_x, q_y, q_z, 1, |q|^2]                (rhs)
  - W^T = Exp(-d2/VAR) on ACT directly from PSUM (the reference's +EPS inside
    the square perturbs the softmax by <1e-4 relative — validated numerically).
  - c^T [C, n] and the softmax denominator accumulate over 32 m-tiles in PSUM
    (lhsT = fea tile / ones column).
  - MLP runs with H on partitions; weights host-pretransposed, layer biases
    folded into the Relu activations as cumulative per-partition bias vectors,
    the residual stream accumulates directly in PSUM (start=False matmuls).

Hardware constraint honored throughout: a Matmult carries at most ONE sem
wait (single LDWEIGHTS wait slot), so ACT is kept the last reader of every
PSUM bank and DMA waits are absorbed by observer instructions.
"""

import numpy as np

import concourse.bass as bass
import concourse.mybir as mybir
from concourse import bacc
import concourse.tile as tile
from concourse.bass_utils import run_bass_kernel_spmd

F32 = mybir.dt.float32
F32R = mybir.dt.float32r
BF16 = mybir.dt.bfloat16
AF = mybir.ActivationFunctionType

N_CORES = 8
N, M, D, C, H = 16384, 4096, 3, 128, 256
NB = 5
NC_PER = N // N_CORES          # 2048 queries per core
CHUNK = 512                    # free-dim tile (one fp32 PSUM bank)
N_CHUNKS = NC_PER // CHUNK     # 4
MT = M // 128                  # 32 context tiles
INV_VAR = 100.0                # 1 / (0.1**2)
MLP_GROUP = 2                  # sub-chunks resident in PSUM per MLP pass

# packed bf16 weight-table column offsets (one DMA for all matmul weights)
OFF_FCC = 0                      # [128, NB*H]   fc_c_W[i].T h-tiles
OFF_B0 = OFF_FCC + NB * H        # [128, NB*2*H] blk0_W[i].T (kt, ht)
OFF_B1 = OFF_B0 + NB * 2 * H
OFF_OW = OFF_B1 + NB * 2 * H     # [128, 2] out_W.T k-tiles
OFF_ONEC = OFF_OW + 2            # [128, 1] ones column (softmax denominator)
WCOLS = OFF_ONEC + 1
# f32 table: relu biases + fc_p weights + broadcast ones row
OFF_BIAS = 0                     # [128, 22] bias vecs (ht*11 + v)
OFF_FCP = OFF_BIAS + 22          # rows 0-2: fc_p_W.T h-tiles
OFF_ONER = OFF_FCP + H           # row 0: ones row (128 cols)
FCOLS = OFF_ONER + 128


def build_bass() -> bass.Bass:
    nc = bacc.Bacc()

    qppp_d = nc.declare_dram_parameter("qppp", [5, NC_PER + M + H], F32R, isOutput=False)
    fea_d = nc.declare_dram_parameter("fea", [M, C], BF16, isOutput=False)
    wtab_d = nc.declare_dram_parameter("wtab", [128, WCOLS], BF16, isOutput=False)
    ftab_d = nc.declare_dram_parameter("ftab", [128, FCOLS], F32, isOutput=False)
    out_d = nc.declare_dram_parameter("out", [1, NC_PER], F32, isOutput=True)

    with tile.TileContext(nc) as tc:
        with (
            tc.tile_pool(name="consts", bufs=1) as consts,
            tc.tile_pool(name="obspsum", bufs=1, space="PSUM") as obs_pool,
        ):
            fea_sb = consts.tile([128, MT, C], BF16, tag="fea")
            nc.sync.dma_start(
                out=fea_sb, in_=fea_d[:, :].rearrange("(t p) c -> p t c", p=128)
            )
            qppp_sb = consts.tile([5, NC_PER + M + H], F32R, tag="qppp")
            nc.sync.dma_start(out=qppp_sb, in_=qppp_d[:, :])
            wt_sb = consts.tile([128, WCOLS], BF16, tag="wtab")
            nc.sync.dma_start(out=wt_sb, in_=wtab_d[:, :])
            ft_sb = consts.tile([128, FCOLS], F32, tag="ftab")
            nc.sync.dma_start(out=ft_sb, in_=ftab_d[:, :])

            qp_sb = qppp_sb[:, 0:NC_PER]
            pp_sb = qppp_sb[:, NC_PER : NC_PER + M]
            fcc_lhsT = lambda i, ht: wt_sb[
                :, OFF_FCC + i * H + ht * 128 : OFF_FCC + i * H + ht * 128 + 128
            ]
            b0_lhsT = lambda i, kt, ht: wt_sb[
                :,
                OFF_B0 + i * 512 + kt * 256 + ht * 128 : OFF_B0
                + i * 512 + kt * 256 + ht * 128 + 128,
            ]
            b1_lhsT = lambda i, kt, ht: wt_sb[
                :,
                OFF_B1 + i * 512 + kt * 256 + ht * 128 : OFF_B1
                + i * 512 + kt * 256 + ht * 128 + 128,
            ]
            fcp_lhsT = lambda ht: qppp_sb[0:D, NC_PER + M + ht * 128 : NC_PER + M + ht * 128 + 128]
            ow_lhsT = lambda kt: wt_sb[:, OFF_OW + kt : OFF_OW + kt + 1]
            bias_ap = lambda ht, v: ft_sb[:, OFF_BIAS + ht * 11 + v : OFF_BIAS + ht * 11 + v + 1]
            ones_col = wt_sb[:, OFF_ONEC : OFF_ONEC + 1]
            ones_row = ft_sb[0:1, OFF_ONER : OFF_ONER + 128]

            cn_sb = consts.tile([C, NC_PER], BF16, tag="cn")  # normalized c^T

            # observer matmul: absorbs the fea DMA wait into the PE clock so
            # the first c^T matmul only waits on ACT
            obs_ps = obs_pool.tile([1, 1], F32, tag="obs")
            nc.tensor.matmul(
                obs_ps, lhsT=fea_sb[0:1, 0, 0:1], rhs=fea_sb[0:1, 0, 0:1],
                start=True, stop=True,
            )

            # ---------------- attention phase ----------------
            with (
                tc.tile_pool(name="spsum", bufs=3, space="PSUM") as s_pool,
                tc.tile_pool(name="ctpsum", bufs=2, space="PSUM") as c_pool,
                tc.tile_pool(name="dnpsum", bufs=2, space="PSUM") as d_pool,
                tc.tile_pool(name="wsb", bufs=3) as w_pool,
                tc.tile_pool(name="rsb", bufs=2) as r_pool,
            ):
                for ch in range(N_CHUNKS):
                    nsl = slice(ch * CHUNK, (ch + 1) * CHUNK)
                    ct_ps = c_pool.tile([C, CHUNK], F32, tag="ct")
                    dn_ps = d_pool.tile([1, CHUNK], F32, tag="dn")

                    def emit_s(t, nsl=nsl):
                        s_ps = s_pool.tile([128, CHUNK], F32, tag="s")
                        nc.tensor.matmul(
                            s_ps,
                            lhsT=pp_sb[:, t * 128 : (t + 1) * 128],
                            rhs=qp_sb[:, nsl],
                            start=True,
                            stop=True,
                        )
                        return s_ps

                    # software-pipeline the d2 matmuls two tiles ahead so the
                    # PE never waits on the ACT exp of the current tile
                    s_tiles = {0: emit_s(0), 1: emit_s(1)}
                    for t in range(MT):
                        w_sb = w_pool.tile([128, CHUNK], BF16, tag="w")
                        nc.scalar.activation(
                            w_sb, s_tiles.pop(t), AF.Exp, scale=-INV_VAR
                        )
                        if t + 2 < MT:
                            s_tiles[t + 2] = emit_s(t + 2)
                        nc.tensor.matmul(
                            ct_ps,
                            lhsT=fea_sb[:, t, :],
                            rhs=w_sb,
                            start=(t == 0),
                            stop=(t == MT - 1),
                        )
                        nc.tensor.matmul(
                            dn_ps,
                            lhsT=ones_col,
                            rhs=w_sb,
                            start=(t == 0),
                            stop=(t == MT - 1),
                        )

                    # normalization: r = 1/denom, broadcast over partitions
                    # via a K=1 ones matmul, then scale c^T.  Every PSUM tile's
                    # last reader stays ACT (single-wait rule).
                    dn_sb = r_pool.tile([1, CHUNK], F32, tag="dnc")
                    nc.scalar.activation(dn_sb, dn_ps, AF.Copy)
                    r_sb = r_pool.tile([1, CHUNK], F32, tag="r")
                    nc.vector.reciprocal(r_sb, dn_sb)
                    rb_ps = s_pool.tile([128, CHUNK], F32, tag="s")
                    nc.tensor.matmul(
                        rb_ps, lhsT=ones_row, rhs=r_sb, start=True, stop=True
                    )
                    rb_sb = r_pool.tile([128, CHUNK], F32, tag="rbc")
                    nc.scalar.activation(rb_sb, rb_ps, AF.Copy)
                    ct_sb = r_pool.tile([C, CHUNK], F32, tag="ctc")
                    nc.scalar.activation(ct_sb, ct_ps, AF.Copy)
                    nc.vector.tensor_mul(cn_sb[:, nsl], ct_sb, rb_sb)

            # ---------------- MLP phase ----------------
            # net^T stays resident in PSUM per (ht, sub); blk1/fc_c matmuls
            # accumulate the residual stream in place.  Two sub-chunks at a
            # time: 2 ht x 2 sub net banks + 3 h banks + obs = 8 banks.
            with (
                tc.tile_pool(name="netpsum", bufs=2 * MLP_GROUP, space="PSUM") as n_pool,
                tc.tile_pool(name="hpsum", bufs=3, space="PSUM") as h_pool,
                tc.tile_pool(name="asb", bufs=4) as a_pool,
                tc.tile_pool(name="bsb", bufs=4) as b_pool,
                tc.tile_pool(name="osb", bufs=2) as os_pool,
            ):
                for g0 in range(0, N_CHUNKS, MLP_GROUP):
                    subs = range(g0, min(g0 + MLP_GROUP, N_CHUNKS))
                    net = {}
                    for sub in subs:
                        nsl = slice(sub * CHUNK, (sub + 1) * CHUNK)
                        for ht in range(2):
                            net_ps = n_pool.tile([128, CHUNK], F32, tag="net")
                            nc.tensor.matmul(
                                net_ps, lhsT=fcp_lhsT(ht), rhs=qp_sb[0:D, nsl],
                                start=True, stop=False,
                            )
                            nc.tensor.matmul(
                                net_ps, lhsT=fcc_lhsT(0, ht), rhs=cn_sb[:, nsl],
                                start=False, stop=True,
                            )
                            net[(ht, sub)] = net_ps

                    for i in range(NB):
                        for sub in subs:
                            nsl = slice(sub * CHUNK, (sub + 1) * CHUNK)
                            rx = []
                            for ht in range(2):
                                rx_sb = a_pool.tile([128, CHUNK], BF16, tag="rx")
                                nc.scalar.activation(
                                    rx_sb, net[(ht, sub)], AF.Relu,
                                    bias=bias_ap(ht, i),
                                )
                                rx.append(rx_sb)
                            h_tiles = []
                            for ht in range(2):
                                h_ps = h_pool.tile([128, CHUNK], F32, tag="h")
                                nc.tensor.matmul(
                                    h_ps, lhsT=b0_lhsT(i, 0, ht), rhs=rx[0],
                                    start=True, stop=False,
                                )
                                nc.tensor.matmul(
                                    h_ps, lhsT=b0_lhsT(i, 1, ht), rhs=rx[1],
                                    start=False, stop=True,
                                )
                                h_tiles.append(h_ps)
                            rh = []
                            for ht in range(2):
                                rh_sb = b_pool.tile([128, CHUNK], BF16, tag="rh")
                                nc.scalar.activation(
                                    rh_sb, h_tiles[ht], AF.Relu,
                                    bias=bias_ap(ht, 6 + i),
                                )
                                rh.append(rh_sb)
                            last = i == NB - 1
                            for ht in range(2):
                                nc.tensor.matmul(
                                    net[(ht, sub)], lhsT=b1_lhsT(i, 0, ht),
                                    rhs=rh[0], start=False, stop=False,
                                    skip_group_check=True,
                                )
                                nc.tensor.matmul(
                                    net[(ht, sub)], lhsT=b1_lhsT(i, 1, ht),
                                    rhs=rh[1], start=False, stop=last,
                                    skip_group_check=True,
                                )
                                if not last:
                                    nc.tensor.matmul(
                                        net[(ht, sub)], lhsT=fcc_lhsT(i + 1, ht),
                                        rhs=cn_sb[:, nsl], start=False, stop=True,
                                        skip_group_check=True,
                                    )

                    # out = out_W @ relu(net + B_y)   (+ out_b added on host)
                    for sub in subs:
                        nsl = slice(sub * CHUNK, (sub + 1) * CHUNK)
                        ry = []
                        for ht in range(2):
                            ry_sb = a_pool.tile([128, CHUNK], BF16, tag="rx")
                            nc.scalar.activation(
                                ry_sb, net[(ht, sub)], AF.Relu, bias=bias_ap(ht, 5)
                            )
                            ry.append(ry_sb)
                        o_ps = h_pool.tile([1, CHUNK], F32, tag="h")
                        nc.tensor.matmul(
                            o_ps, lhsT=ow_lhsT(0), rhs=ry[0], start=True, stop=False
                        )
                        nc.tensor.matmul(
                            o_ps, lhsT=ow_lhsT(1), rhs=ry[1], start=False, stop=True
                        )
                        out_sb = os_pool.tile([1, CHUNK], F32, tag="osb")
                        nc.scalar.activation(out_sb, o_ps, AF.Copy)
                        nc.sync.dma_start(out=out_d[:, nsl], in_=out_sb)

    return nc


def host_prep(inputs):
    p = np.asarray(inputs["p"], np.float32)[0]      # [N, 3]
    pp = np.asarray(inputs["pp"], np.float32)[0]    # [M, 3]
    fea = np.ascontiguousarray(np.asarray(inputs["fea"], np.float32)[0])  # [M, C]

    qpack = np.empty((5, N), np.float32)
    qpack[0:3] = p.T
    qpack[3] = 1.0
    qpack[4] = (p * p).sum(1)
    ppack = np.empty((5, M), np.float32)
    ppack[0:3] = -2.0 * pp.T
    ppack[3] = (pp * pp).sum(1)
    ppack[4] = 1.0

    fc_p_W = np.asarray(inputs["fc_p_W"], np.float32)    # [H, 3]
    fc_c_W = np.asarray(inputs["fc_c_W"], np.float32)    # [NB, H, C]
    blk0_W = np.asarray(inputs["blk0_W"], np.float32)    # [NB, H, H]
    blk1_W = np.asarray(inputs["blk1_W"], np.float32)
    out_W = np.asarray(inputs["out_W"], np.float32)      # [1, H]
    fc_p_b = np.asarray(inputs["fc_p_b"], np.float32)
    fc_c_b = np.asarray(inputs["fc_c_b"], np.float32)    # [NB, H]
    blk0_b = np.asarray(inputs["blk0_b"], np.float32)
    blk1_b = np.asarray(inputs["blk1_b"], np.float32)
    out_b = float(np.asarray(inputs["out_b"], np.float32)[0])

    import ml_dtypes
    wtab = np.zeros((128, WCOLS), ml_dtypes.bfloat16)
    wtab[:, OFF_FCC : OFF_FCC + NB * H] = fc_c_W.transpose(2, 0, 1).reshape(C, NB * H)
    wtab[:, OFF_B0 : OFF_B0 + NB * 2 * H] = (
        blk0_W.reshape(NB, H, 2, 128).transpose(3, 0, 2, 1).reshape(128, NB * 2 * H)
    )
    wtab[:, OFF_B1 : OFF_B1 + NB * 2 * H] = (
        blk1_W.reshape(NB, H, 2, 128).transpose(3, 0, 2, 1).reshape(128, NB * 2 * H)
    )
    wtab[:, OFF_OW : OFF_OW + 2] = out_W.reshape(2, 128).T
    wtab[:, OFF_ONEC] = 1.0
    ftab = np.zeros((128, FCOLS), np.float32)

    # cumulative bias vectors folded into the Relu activations:
    #   vec 0..4  = B_i  (bias of net before block i's first relu)
    #   vec 5     = B_y  (bias of net before the final relu)
    #   vec 6..10 = blk0_b[i]  (bias of h before block i's second relu)
    vecs = np.zeros((11, H), np.float32)
    B = fc_p_b + fc_c_b[0]
    for i in range(NB):
        vecs[i] = B
        vecs[6 + i] = blk0_b[i]
        B = B + blk1_b[i] + (fc_c_b[i + 1] if i + 1 < NB else 0.0)
    vecs[5] = B
    ftab[:, OFF_BIAS : OFF_BIAS + 22] = (
        vecs.reshape(11, 2, 128).transpose(2, 1, 0).reshape(128, 22)
    )
    ftab[0, OFF_ONER : OFF_ONER + 128] = 1.0

    shared = {"fea": fea.astype(ml_dtypes.bfloat16), "wtab": wtab, "ftab": ftab}
    in_maps = []
    for c in range(N_CORES):
        m = dict(shared)
        qppp = np.zeros((5, NC_PER + M + H), np.float32)
        qppp[:, 0:NC_PER] = qpack[:, c * NC_PER : (c + 1) * NC_PER]
        qppp[:, NC_PER : NC_PER + M] = ppack
        qppp[0:D, NC_PER + M :] = fc_p_W.T
        m["qppp"] = qppp
        in_maps.append(m)
    return in_maps, out_b


_NC_CACHE = {}


def kernel(**inputs) -> np.ndarray:
    in_maps, out_b = host_prep(inputs)
    if "nc" not in _NC_CACHE:
        nc = build_bass()
        nc.finalize()
        _NC_CACHE["nc"] = nc
    nc = _NC_CACHE["nc"]
    res = run_bass_kernel_spmd(nc, in_maps, list(range(N_CORES)))
    parts = [res.results[c]["out"] for c in range(N_CORES)]
    out = np.concatenate(parts, axis=1).astype(np.float32) + np.float32(out_b)
    return out


# revision 17
# speedup vs baseline: 2.6135x; 1.2989x over previous
"""Trainium2 Bass kernel for nn_LocalPointDecoder (sparse_attention).

Algorithm (per query point n):
  c[n]  = softmax_m(-|q_n - pp_m|^2 / VAR) @ fea          (Gaussian point attention)
  out[n] = MLP(c[n], q_n)                                  (5-block ResNet MLP, H=256)

Sharding: query points N=16384 split across 8 cores (2048 each); every core
holds the full pp/fea context and all MLP weights.

Device layout is fully transposed (features on partitions, queries on the
free axis):
  - d2^T [m, n] from ONE K=5 matmul of host-packed quadratic-form operands:
      ppack = [-2*pp_x, -2*pp_y, -2*pp_z, |pp|^2, 1]   (lhsT)
      qpack = [q_x, q_y, q_z, 1, |q|^2]                (rhs)
  - W^T = Exp(-d2/VAR) on ACT directly from PSUM (the reference's +EPS inside
    the square perturbs the softmax by <1e-4 relative — validated numerically).
  - c^T [C, n] and the softmax denominator accumulate over 32 m-tiles in PSUM
    (lhsT = fea tile / ones column).
  - MLP runs with H on partitions; weights host-pretransposed, layer biases
    folded into the Relu activations as cumulative per-partition bias vectors,
    the residual stream accumulates directly in PSUM (start=False matmuls).

Hardware constraint honored throughout: a Matmult carries at most ONE sem
wait (single LDWEIGHTS wait slot), so ACT is kept the last reader of every
PSUM bank and DMA waits are absorbed by observer instructions.
"""

import numpy as np

import concourse.bass as bass
import concourse.mybir as mybir
from concourse import bacc
import concourse.tile as tile
from concourse.bass_utils import run_bass_kernel_spmd

F32 = mybir.dt.float32
F32R = mybir.dt.float32r
BF16 = mybir.dt.bfloat16
AF = mybir.ActivationFunctionType

N_CORES = 8
N, M, D, C, H = 16384, 4096, 3, 128, 256
NB = 5
NC_PER = N // N_CORES          # 2048 queries per core
CHUNK = 512                    # free-dim tile (one fp32 PSUM bank)
N_CHUNKS = NC_PER // CHUNK     # 4
MT = M // 128                  # 32 context tiles
INV_VAR = 100.0                # 1 / (0.1**2)
MLP_GROUP = 2                  # sub-chunks resident in PSUM per MLP pass

# packed bf16 weight-table column offsets (one DMA for all matmul weights)
OFF_FCC = 0                      # [128, NB*H]   fc_c_W[i].T h-tiles
OFF_B0 = OFF_FCC + NB * H        # [128, NB*2*H] blk0_W[i].T (kt, ht)
OFF_B1 = OFF_B0 + NB * 2 * H
OFF_OW = OFF_B1 + NB * 2 * H     # [128, 2] out_W.T k-tiles
OFF_ONEC = OFF_OW + 2            # [128, 1] ones column (softmax denominator)
WCOLS = OFF_ONEC + 1
# f32 table: relu biases + fc_p weights + broadcast ones row
OFF_BIAS = 0                     # [128, 22] bias vecs (ht*11 + v)
OFF_FCP = OFF_BIAS + 22          # rows 0-2: fc_p_W.T h-tiles
OFF_ONER = OFF_FCP + H           # row 0: ones row (128 cols)
OFF_ONECF = OFF_ONER + 128       # [128, 1] f32 ones column (denom reduce)
FCOLS = OFF_ONECF + 1


def build_bass() -> bass.Bass:
    nc = bacc.Bacc()

    q4_d = nc.declare_dram_parameter("q4", [128, NC_PER], F32, isOutput=False)
    pp4_d = nc.declare_dram_parameter("pp4", [128, (MT // 4) * 128], F32, isOutput=False)
    fea_d = nc.declare_dram_parameter("fea", [M, C], BF16, isOutput=False)
    wtab_d = nc.declare_dram_parameter("wtab", [128, WCOLS], BF16, isOutput=False)
    ftab_d = nc.declare_dram_parameter("ftab", [128, FCOLS], F32, isOutput=False)
    out_d = nc.declare_dram_parameter("out", [1, NC_PER], F32, isOutput=True)
    import os
    dbg_d = None
    if os.environ.get("K_DEBUG"):
        dbg_d = nc.declare_dram_parameter("dbg", [C, NC_PER], BF16, isOutput=True)

    with tile.TileContext(nc) as tc:
        with tc.tile_pool(name="consts", bufs=1) as consts:
            fea_sb = consts.tile([128, MT, C], BF16, tag="fea")
            nc.sync.dma_start(
                out=fea_sb, in_=fea_d[:, :].rearrange("(t p) c -> p t c", p=128)
            )
            q4_sb = consts.tile([128, NC_PER], F32, tag="q4")
            nc.sync.dma_start(out=q4_sb, in_=q4_d[:, :])
            pp4_sb = consts.tile([128, (MT // 4) * 128], F32, tag="pp4")
            nc.sync.dma_start(out=pp4_sb, in_=pp4_d[:, :])
            wt_sb = consts.tile([128, WCOLS], BF16, tag="wtab")
            nc.sync.dma_start(out=wt_sb, in_=wtab_d[:, :])
            ft_sb = consts.tile([128, FCOLS], F32, tag="ftab")
            nc.sync.dma_start(out=ft_sb, in_=ftab_d[:, :])

            fcc_lhsT = lambda i, ht: wt_sb[
                :, OFF_FCC + i * H + ht * 128 : OFF_FCC + i * H + ht * 128 + 128
            ]
            b0_lhsT = lambda i, kt, ht: wt_sb[
                :,
                OFF_B0 + i * 512 + kt * 256 + ht * 128 : OFF_B0
                + i * 512 + kt * 256 + ht * 128 + 128,
            ]
            b1_lhsT = lambda i, kt, ht: wt_sb[
                :,
                OFF_B1 + i * 512 + kt * 256 + ht * 128 : OFF_B1
                + i * 512 + kt * 256 + ht * 128 + 128,
            ]
            fcp_lhsT = lambda ht: ft_sb[0:D, OFF_FCP + ht * 128 : OFF_FCP + ht * 128 + 128]
            ow_lhsT = lambda kt: wt_sb[:, OFF_OW + kt : OFF_OW + kt + 1]
            bias_ap = lambda ht, v: ft_sb[:, OFF_BIAS + ht * 11 + v : OFF_BIAS + ht * 11 + v + 1]
            ones_col = wt_sb[:, OFF_ONEC : OFF_ONEC + 1]
            ones_row = ft_sb[0:1, OFF_ONER : OFF_ONER + 128]
            ones_col_f = ft_sb[:, OFF_ONECF : OFF_ONECF + 1]

            cn_sb = consts.tile([C, NC_PER], BF16, tag="cn")  # normalized c^T

            # ---------------- attention phase ----------------
            with (
                tc.tile_pool(name="spsum", bufs=5, space="PSUM") as s_pool,
                tc.tile_pool(name="ctpsum", bufs=2, space="PSUM") as c_pool,
                tc.tile_pool(name="dnpsum", bufs=1, space="PSUM") as d_pool,
                tc.tile_pool(name="wsb", bufs=6) as w_pool,
                tc.tile_pool(name="accsb", bufs=2) as a2_pool,
                tc.tile_pool(name="rsb", bufs=2) as r_pool,
            ):
                NG = MT // 4  # 8 groups of 4 row-tiled d2 matmuls
                for ch in range(N_CHUNKS):
                    nsl = slice(ch * CHUNK, (ch + 1) * CHUNK)
                    ct_ps = c_pool.tile([C, CHUNK], F32, tag="ct")

                    def emit_s(j, nsl=nsl):
                        # 4 concurrent K=5 matmuls in distinct PE row groups
                        tiles = []
                        for g in range(4):
                            s_ps = s_pool.tile([128, CHUNK], F32, tag="s")
                            nc.tensor.matmul(
                                s_ps,
                                lhsT=pp4_sb[32 * g : 32 * g + 5,
                                            j * 128 : (j + 1) * 128],
                                rhs=q4_sb[32 * g : 32 * g + 5, nsl],
                                start=True,
                                stop=True,
                                tile_position=(32 * g, 0),
                            )
                            tiles.append(s_ps)
                        return tiles

                    s_tiles = {0: emit_s(0)}
                    if ch == 0:
                        # absorb the fea DMA wait into PE before the first
                        # c^T matmul (fea loads while the d2 matmuls run)
                        obs_ps = s_pool.tile([1, 1], F32, tag="s")
                        nc.tensor.matmul(
                            obs_ps, lhsT=fea_sb[0:1, 0, 0:1],
                            rhs=fea_sb[0:1, 0, 0:1], start=True, stop=True,
                        )
                    dn_acc = a2_pool.tile([128, CHUNK], F32, tag="dnacc")
                    for j in range(NG):
                        group = s_tiles.pop(j)
                        ws = []
                        for g in range(4):
                            w_sb = w_pool.tile([128, CHUNK], BF16, tag="w")
                            nc.scalar.activation(
                                w_sb, group[g], AF.Exp, scale=-INV_VAR
                            )
                            ws.append(w_sb)
                        if j + 1 < NG:
                            s_tiles[j + 1] = emit_s(j + 1)
                        for g in range(4):
                            t = 4 * j + g
                            nc.tensor.matmul(
                                ct_ps,
                                lhsT=fea_sb[:, t, :],
                                rhs=ws[g],
                                start=(t == 0),
                                stop=(t == MT - 1),
                            )
                            # denominator accumulates on the (idle) DVE
                            if t == 0:
                                nc.vector.tensor_copy(dn_acc, ws[g])
                            else:
                                nc.vector.tensor_add(dn_acc, dn_acc, ws[g])

                    # partition-reduce the DVE accumulator, then r = 1/denom,
                    # broadcast via a K=1 ones matmul, scale c^T.  Every PSUM
                    # tile's last reader stays ACT (single-wait rule).
                    dn_ps = d_pool.tile([1, CHUNK], F32, tag="dn")
                    nc.tensor.matmul(
                        dn_ps, lhsT=ones_col_f, rhs=dn_acc, start=True, stop=True
                    )
                    dn_sb = r_pool.tile([1, CHUNK], F32, tag="dnc")
                    nc.scalar.activation(dn_sb, dn_ps, AF.Copy)
                    r_sb = r_pool.tile([1, CHUNK], F32, tag="r")
                    nc.vector.reciprocal(r_sb, dn_sb)
                    rb_ps = s_pool.tile([128, CHUNK], F32, tag="s")
                    nc.tensor.matmul(
                        rb_ps, lhsT=ones_row, rhs=r_sb, start=True, stop=True
                    )
                    rb_sb = r_pool.tile([128, CHUNK], F32, tag="rbc")
                    nc.scalar.activation(rb_sb, rb_ps, AF.Copy)
                    ct_sb = r_pool.tile([C, CHUNK], F32, tag="ctc")
                    nc.scalar.activation(ct_sb, ct_ps, AF.Copy)
                    nc.vector.tensor_mul(cn_sb[:, nsl], ct_sb, rb_sb)

            if dbg_d is not None:
                nc.sync.dma_start(out=dbg_d[:, :], in_=cn_sb)

            # ---------------- MLP phase ----------------
            # net^T stays resident in PSUM per (ht, sub); blk1/fc_c matmuls
            # accumulate the residual stream in place.  Two sub-chunks at a
            # time: 2 ht x 2 sub net banks + 3 h banks + obs = 8 banks.
            with (
                tc.tile_pool(name="netpsum", bufs=2 * MLP_GROUP, space="PSUM") as n_pool,
                tc.tile_pool(name="hpsum", bufs=3, space="PSUM") as h_pool,
                tc.tile_pool(name="asb", bufs=4) as a_pool,
                tc.tile_pool(name="bsb", bufs=4) as b_pool,
                tc.tile_pool(name="osb", bufs=2) as os_pool,
            ):
                for g0 in range(0, N_CHUNKS, MLP_GROUP):
                    subs = range(g0, min(g0 + MLP_GROUP, N_CHUNKS))
                    net = {}
                    for sub in subs:
                        nsl = slice(sub * CHUNK, (sub + 1) * CHUNK)
                        for ht in range(2):
                            net_ps = n_pool.tile([128, CHUNK], F32, tag="net")
                            nc.tensor.matmul(
                                net_ps, lhsT=fcp_lhsT(ht), rhs=q4_sb[0:D, nsl],
                                start=True, stop=False,
                            )
                            nc.tensor.matmul(
                                net_ps, lhsT=fcc_lhsT(0, ht), rhs=cn_sb[:, nsl],
                                start=False, stop=True,
                            )
                            net[(ht, sub)] = net_ps

                    for i in range(NB):
                        for sub in subs:
                            nsl = slice(sub * CHUNK, (sub + 1) * CHUNK)
                            rx = []
                            for ht in range(2):
                                rx_sb = a_pool.tile([128, CHUNK], BF16, tag="rx")
                                nc.scalar.activation(
                                    rx_sb, net[(ht, sub)], AF.Relu,
                                    bias=bias_ap(ht, i),
                                )
                                rx.append(rx_sb)
                            h_tiles = []
                            for ht in range(2):
                                h_ps = h_pool.tile([128, CHUNK], F32, tag="h")
                                nc.tensor.matmul(
                                    h_ps, lhsT=b0_lhsT(i, 0, ht), rhs=rx[0],
                                    start=True, stop=False,
                                )
                                nc.tensor.matmul(
                                    h_ps, lhsT=b0_lhsT(i, 1, ht), rhs=rx[1],
                                    start=False, stop=True,
                                )
                                h_tiles.append(h_ps)
                            rh = []
                            for ht in range(2):
                                rh_sb = b_pool.tile([128, CHUNK], BF16, tag="rh")
                                nc.scalar.activation(
                                    rh_sb, h_tiles[ht], AF.Relu,
                                    bias=bias_ap(ht, 6 + i),
                                )
                                rh.append(rh_sb)
                            last = i == NB - 1
                            for ht in range(2):
                                nc.tensor.matmul(
                                    net[(ht, sub)], lhsT=b1_lhsT(i, 0, ht),
                                    rhs=rh[0], start=False, stop=False,
                                    skip_group_check=True,
                                )
                                nc.tensor.matmul(
                                    net[(ht, sub)], lhsT=b1_lhsT(i, 1, ht),
                                    rhs=rh[1], start=False, stop=last,
                                    skip_group_check=True,
                                )
                                if not last:
                                    nc.tensor.matmul(
                                        net[(ht, sub)], lhsT=fcc_lhsT(i + 1, ht),
                                        rhs=cn_sb[:, nsl], start=False, stop=True,
                                        skip_group_check=True,
                                    )

                    # out = out_W @ relu(net + B_y)   (+ out_b added on host)
                    for sub in subs:
                        nsl = slice(sub * CHUNK, (sub + 1) * CHUNK)
                        ry = []
                        for ht in range(2):
                            ry_sb = a_pool.tile([128, CHUNK], BF16, tag="rx")
                            nc.scalar.activation(
                                ry_sb, net[(ht, sub)], AF.Relu, bias=bias_ap(ht, 5)
                            )
                            ry.append(ry_sb)
                        o_ps = h_pool.tile([1, CHUNK], F32, tag="h")
                        nc.tensor.matmul(
                            o_ps, lhsT=ow_lhsT(0), rhs=ry[0], start=True, stop=False
                        )
                        nc.tensor.matmul(
                            o_ps, lhsT=ow_lhsT(1), rhs=ry[1], start=False, stop=True
                        )
                        out_sb = os_pool.tile([1, CHUNK], F32, tag="osb")
                        nc.scalar.activation(out_sb, o_ps, AF.Copy)
                        nc.sync.dma_start(out=out_d[:, nsl], in_=out_sb)

    return nc


def host_prep(inputs):
    p = np.asarray(inputs["p"], np.float32)[0]      # [N, 3]
    pp = np.asarray(inputs["pp"], np.float32)[0]    # [M, 3]
    fea = np.ascontiguousarray(np.asarray(inputs["fea"], np.float32)[0])  # [M, C]

    qpack = np.empty((5, N), np.float32)
    qpack[0:3] = p.T
    qpack[3] = 1.0
    qpack[4] = (p * p).sum(1)
    ppack = np.empty((5, M), np.float32)
    ppack[0:3] = -2.0 * pp.T
    ppack[3] = (pp * pp).sum(1)
    ppack[4] = 1.0

    fc_p_W = np.asarray(inputs["fc_p_W"], np.float32)    # [H, 3]
    fc_c_W = np.asarray(inputs["fc_c_W"], np.float32)    # [NB, H, C]
    blk0_W = np.asarray(inputs["blk0_W"], np.float32)    # [NB, H, H]
    blk1_W = np.asarray(inputs["blk1_W"], np.float32)
    out_W = np.asarray(inputs["out_W"], np.float32)      # [1, H]
    fc_p_b = np.asarray(inputs["fc_p_b"], np.float32)
    fc_c_b = np.asarray(inputs["fc_c_b"], np.float32)    # [NB, H]
    blk0_b = np.asarray(inputs["blk0_b"], np.float32)
    blk1_b = np.asarray(inputs["blk1_b"], np.float32)
    out_b = float(np.asarray(inputs["out_b"], np.float32)[0])

    import ml_dtypes
    wtab = np.zeros((128, WCOLS), ml_dtypes.bfloat16)
    wtab[:, OFF_FCC : OFF_FCC + NB * H] = fc_c_W.transpose(2, 0, 1).reshape(C, NB * H)
    wtab[:, OFF_B0 : OFF_B0 + NB * 2 * H] = (
        blk0_W.reshape(NB, H, 2, 128).transpose(3, 0, 2, 1).reshape(128, NB * 2 * H)
    )
    wtab[:, OFF_B1 : OFF_B1 + NB * 2 * H] = (
        blk1_W.reshape(NB, H, 2, 128).transpose(3, 0, 2, 1).reshape(128, NB * 2 * H)
    )
    wtab[:, OFF_OW : OFF_OW + 2] = out_W.reshape(2, 128).T
    wtab[:, OFF_ONEC] = 1.0
    ftab = np.zeros((128, FCOLS), np.float32)

    # cumulative bias vectors folded into the Relu activations:
    #   vec 0..4  = B_i  (bias of net before block i's first relu)
    #   vec 5     = B_y  (bias of net before the final relu)
    #   vec 6..10 = blk0_b[i]  (bias of h before block i's second relu)
    vecs = np.zeros((11, H), np.float32)
    B = fc_p_b + fc_c_b[0]
    for i in range(NB):
        vecs[i] = B
        vecs[6 + i] = blk0_b[i]
        B = B + blk1_b[i] + (fc_c_b[i + 1] if i + 1 < NB else 0.0)
    vecs[5] = B
    ftab[:, OFF_BIAS : OFF_BIAS + 22] = (
        vecs.reshape(11, 2, 128).transpose(2, 1, 0).reshape(128, 22)
    )
    ftab[0, OFF_ONER : OFF_ONER + 128] = 1.0

    ftab[:, OFF_ONECF] = 1.0
    ftab[0:D, OFF_FCP : OFF_FCP + H] = fc_p_W.T
    # pp4: tile 4j+g lives at partitions 32g..32g+5, columns j*128..(j+1)*128
    pp4 = np.zeros((128, (M // 512) * 128), np.float32)
    for j in range(M // 512):
        for g in range(4):
            t = 4 * j + g
            pp4[32 * g : 32 * g + 5, j * 128 : (j + 1) * 128] = (
                ppack[:, t * 128 : (t + 1) * 128]
            )
    shared = {"fea": fea.astype(ml_dtypes.bfloat16), "wtab": wtab, "ftab": ftab,
              "pp4": pp4}
    in_maps = []
    for c in range(N_CORES):
        m = dict(shared)
        qc = qpack[:, c * NC_PER : (c + 1) * NC_PER]
        q4 = np.zeros((128, NC_PER), np.float32)
        for g in range(4):
            q4[32 * g : 32 * g + 5, :] = qc
        m["q4"] = q4
        in_maps.append(m)
    return in_maps, out_b


_NC_CACHE = {}


def kernel(**inputs) -> np.ndarray:
    in_maps, out_b = host_prep(inputs)
    if "nc" not in _NC_CACHE:
        nc = build_bass()
        nc.finalize()
        _NC_CACHE["nc"] = nc
    nc = _NC_CACHE["nc"]
    res = run_bass_kernel_spmd(nc, in_maps, list(range(N_CORES)))
    parts = [res.results[c]["out"] for c in range(N_CORES)]
    out = np.concatenate(parts, axis=1).astype(np.float32) + np.float32(out_b)
    return out


# revision 20
# speedup vs baseline: 2.7143x; 1.0386x over previous
"""Trainium2 Bass kernel for nn_LocalPointDecoder (sparse_attention).

Algorithm (per query point n):
  c[n]  = softmax_m(-|q_n - pp_m|^2 / VAR) @ fea          (Gaussian point attention)
  out[n] = MLP(c[n], q_n)                                  (5-block ResNet MLP, H=256)

Sharding: query points N=16384 split across 8 cores (2048 each); every core
holds the full pp/fea context and all MLP weights.

Device layout is fully transposed (features on partitions, queries on the
free axis):
  - d2^T [m, n] from ONE K=5 matmul of host-packed quadratic-form operands:
      ppack = [-2*pp_x, -2*pp_y, -2*pp_z, |pp|^2, 1]   (lhsT)
      qpack = [q_x, q_y, q_z, 1, |q|^2]                (rhs)
  - W^T = Exp(-d2/VAR) on ACT directly from PSUM (the reference's +EPS inside
    the square perturbs the softmax by <1e-4 relative — validated numerically).
  - c^T [C, n] and the softmax denominator accumulate over 32 m-tiles in PSUM
    (lhsT = fea tile / ones column).
  - MLP runs with H on partitions; weights host-pretransposed, layer biases
    folded into the Relu activations as cumulative per-partition bias vectors,
    the residual stream accumulates directly in PSUM (start=False matmuls).

Hardware constraint honored throughout: a Matmult carries at most ONE sem
wait (single LDWEIGHTS wait slot), so ACT is kept the last reader of every
PSUM bank and DMA waits are absorbed by observer instructions.
"""

import numpy as np

import concourse.bass as bass
import concourse.mybir as mybir
from concourse import bacc
import concourse.tile as tile
from concourse.bass_utils import run_bass_kernel_spmd

F32 = mybir.dt.float32
F32R = mybir.dt.float32r
BF16 = mybir.dt.bfloat16
AF = mybir.ActivationFunctionType

N_CORES = 8
N, M, D, C, H = 16384, 4096, 3, 128, 256
NB = 5
NC_PER = N // N_CORES          # 2048 queries per core
CHUNK = 512                    # free-dim tile (one fp32 PSUM bank)
N_CHUNKS = NC_PER // CHUNK     # 4
MT = M // 128                  # 32 context tiles
INV_VAR = 100.0                # 1 / (0.1**2)
MLP_GROUP = 2                  # sub-chunks resident in PSUM per MLP pass

# packed bf16 weight-table column offsets (one DMA for all matmul weights)
OFF_FCC = 0                      # [128, NB*H]   fc_c_W[i].T h-tiles
OFF_B0 = OFF_FCC + NB * H        # [128, NB*2*H] blk0_W[i].T (kt, ht)
OFF_B1 = OFF_B0 + NB * 2 * H
OFF_OW = OFF_B1 + NB * 2 * H     # [128, 2] out_W.T k-tiles
OFF_ONEC = OFF_OW + 2            # [128, 1] ones column (softmax denominator)
WCOLS = OFF_ONEC + 1
# f32 table: relu biases + fc_p weights + broadcast ones row
OFF_BIAS = 0                     # [128, 22] bias vecs (ht*11 + v)
OFF_FCP = OFF_BIAS + 22          # rows 0-2: fc_p_W.T h-tiles
OFF_ONER = OFF_FCP + H           # row 0: ones row (128 cols)
OFF_ONECF = OFF_ONER + 128       # [128, 1] f32 ones column (denom reduce)
FCOLS = OFF_ONECF + 1


def build_bass() -> bass.Bass:
    nc = bacc.Bacc()

    q4_d = nc.declare_dram_parameter("q4", [20, NC_PER], F32, isOutput=False)
    pp4_d = nc.declare_dram_parameter("pp4", [20, (MT // 4) * 128], F32, isOutput=False)
    fea_d = nc.declare_dram_parameter("fea", [M, C], BF16, isOutput=False)
    wtab_d = nc.declare_dram_parameter("wtab", [128, WCOLS], BF16, isOutput=False)
    ftab_d = nc.declare_dram_parameter("ftab", [128, FCOLS], F32, isOutput=False)
    out_d = nc.declare_dram_parameter("out", [1, NC_PER], F32, isOutput=True)
    import os
    dbg_d = None
    if os.environ.get("K_DEBUG"):
        dbg_d = nc.declare_dram_parameter("dbg", [C, NC_PER], BF16, isOutput=True)

    with tile.TileContext(nc) as tc:
        with tc.tile_pool(name="consts", bufs=1) as consts:
            fea_sb = consts.tile([128, MT, C], BF16, tag="fea")
            nc.sync.dma_start(
                out=fea_sb, in_=fea_d[:, :].rearrange("(t p) c -> p t c", p=128)
            )
            q4_sb = consts.tile([128, NC_PER], F32, tag="q4")
            pp4_sb = consts.tile([128, (MT // 4) * 128], F32, tag="pp4")
            q20_sb = consts.tile([20, NC_PER], F32, tag="q20")
            nc.sync.dma_start(out=q20_sb, in_=q4_d[:, :])
            pp20_sb = consts.tile([20, (MT // 4) * 128], F32, tag="pp20")
            nc.sync.dma_start(out=pp20_sb, in_=pp4_d[:, :])
            for g in range(4):
                nc.sync.dma_start(
                    out=q4_sb[32 * g : 32 * g + 5, :],
                    in_=q20_sb[5 * g : 5 * g + 5, :],
                )
                nc.sync.dma_start(
                    out=pp4_sb[32 * g : 32 * g + 5, :],
                    in_=pp20_sb[5 * g : 5 * g + 5, :],
                )
            wt_sb = consts.tile([128, WCOLS], BF16, tag="wtab")
            nc.sync.dma_start(out=wt_sb, in_=wtab_d[:, :])
            ft_sb = consts.tile([128, FCOLS], F32, tag="ftab")
            nc.sync.dma_start(out=ft_sb, in_=ftab_d[:, :])

            fcc_lhsT = lambda i, ht: wt_sb[
                :, OFF_FCC + i * H + ht * 128 : OFF_FCC + i * H + ht * 128 + 128
            ]
            b0_lhsT = lambda i, kt, ht: wt_sb[
                :,
                OFF_B0 + i * 512 + kt * 256 + ht * 128 : OFF_B0
                + i * 512 + kt * 256 + ht * 128 + 128,
            ]
            b1_lhsT = lambda i, kt, ht: wt_sb[
                :,
                OFF_B1 + i * 512 + kt * 256 + ht * 128 : OFF_B1
                + i * 512 + kt * 256 + ht * 128 + 128,
            ]
            fcp_lhsT = lambda ht: ft_sb[0:D, OFF_FCP + ht * 128 : OFF_FCP + ht * 128 + 128]
            ow_lhsT = lambda kt: wt_sb[:, OFF_OW + kt : OFF_OW + kt + 1]
            bias_ap = lambda ht, v: ft_sb[:, OFF_BIAS + ht * 11 + v : OFF_BIAS + ht * 11 + v + 1]
            ones_col = wt_sb[:, OFF_ONEC : OFF_ONEC + 1]
            ones_row = ft_sb[0:1, OFF_ONER : OFF_ONER + 128]
            ones_col_f = ft_sb[:, OFF_ONECF : OFF_ONECF + 1]

            cn_sb = consts.tile([C, NC_PER], BF16, tag="cn")  # normalized c^T

            # ---------------- attention phase ----------------
            r_pool_cm = tc.tile_pool(name="rsb", bufs=4)
            r_pool = r_pool_cm.__enter__()
            with (
                tc.tile_pool(name="spsum", bufs=5, space="PSUM") as s_pool,
                tc.tile_pool(name="ctpsum", bufs=2, space="PSUM") as c_pool,
                tc.tile_pool(name="dnpsum", bufs=1, space="PSUM") as d_pool,
                tc.tile_pool(name="wsb", bufs=6) as w_pool,
                tc.tile_pool(name="accsb", bufs=2) as a2_pool,
            ):
                NG = MT // 4  # 8 groups of 4 row-tiled d2 matmuls
                chunk_norm = []
                for ch in range(N_CHUNKS):
                    nsl = slice(ch * CHUNK, (ch + 1) * CHUNK)
                    ct_ps = c_pool.tile([C, CHUNK], F32, tag="ct")

                    def emit_s(j, nsl=nsl):
                        # 4 concurrent K=5 matmuls in distinct PE row groups
                        tiles = []
                        for g in range(4):
                            s_ps = s_pool.tile([128, CHUNK], F32, tag="s")
                            nc.tensor.matmul(
                                s_ps,
                                lhsT=pp4_sb[32 * g : 32 * g + 5,
                                            j * 128 : (j + 1) * 128],
                                rhs=q4_sb[32 * g : 32 * g + 5, nsl],
                                start=True,
                                stop=True,
                                tile_position=(32 * g, 0),
                            )
                            tiles.append(s_ps)
                        return tiles

                    s_tiles = {0: emit_s(0)}
                    if ch == 0:
                        # absorb the fea DMA wait into PE before the first
                        # c^T matmul (fea loads while the d2 matmuls run)
                        obs_ps = s_pool.tile([1, 1], F32, tag="s")
                        nc.tensor.matmul(
                            obs_ps, lhsT=fea_sb[0:1, 0, 0:1],
                            rhs=fea_sb[0:1, 0, 0:1], start=True, stop=True,
                        )
                    dn_acc = a2_pool.tile([128, CHUNK], F32, tag="dnacc")
                    for j in range(NG):
                        group = s_tiles.pop(j)
                        ws = []
                        for g in range(4):
                            w_sb = w_pool.tile([128, CHUNK], BF16, tag="w")
                            nc.scalar.activation(
                                w_sb, group[g], AF.Exp, scale=-INV_VAR
                            )
                            ws.append(w_sb)
                        if j + 1 < NG:
                            s_tiles[j + 1] = emit_s(j + 1)
                        for g in range(4):
                            t = 4 * j + g
                            nc.tensor.matmul(
                                ct_ps,
                                lhsT=fea_sb[:, t, :],
                                rhs=ws[g],
                                start=(t == 0),
                                stop=(t == MT - 1),
                            )
                            # denominator accumulates on the (idle) DVE
                            if t == 0:
                                nc.vector.tensor_copy(dn_acc, ws[g])
                            else:
                                nc.vector.tensor_add(dn_acc, dn_acc, ws[g])

                    # partition-reduce the DVE accumulator, then r = 1/denom,
                    # broadcast via a K=1 ones matmul, scale c^T.  Every PSUM
                    # tile's last reader stays ACT (single-wait rule).
                    dn_ps = d_pool.tile([1, CHUNK], F32, tag="dn")
                    nc.tensor.matmul(
                        dn_ps, lhsT=ones_col_f, rhs=dn_acc, start=True, stop=True
                    )
                    dn_sb = r_pool.tile([1, CHUNK], F32, tag="dnc")
                    nc.scalar.activation(dn_sb, dn_ps, AF.Copy)
                    r_sb = r_pool.tile([1, CHUNK], F32, tag="r")
                    nc.vector.reciprocal(r_sb, dn_sb)
                    ct_sb = r_pool.tile([C, CHUNK], F32, tag="ctc")
                    nc.scalar.activation(ct_sb, ct_ps, AF.Copy)
                    chunk_norm.append((nsl, r_sb, ct_sb))

            if dbg_d is not None:
                nc.sync.dma_start(out=dbg_d[:, :], in_=cn_sb)

            # ---------------- MLP phase ----------------
            # net^T stays resident in PSUM per (ht, sub); blk1/fc_c matmuls
            # accumulate the residual stream in place.  Two sub-chunks at a
            # time: 2 ht x 2 sub net banks + 3 h banks + obs = 8 banks.
            with (
                tc.tile_pool(name="netpsum", bufs=2 * MLP_GROUP, space="PSUM") as n_pool,
                tc.tile_pool(name="hpsum", bufs=3, space="PSUM") as h_pool,
                tc.tile_pool(name="asb", bufs=4) as a_pool,
                tc.tile_pool(name="bsb", bufs=4) as b_pool,
                tc.tile_pool(name="osb", bufs=2) as os_pool,
            ):
                for nsl_c, r_c, ct_c in chunk_norm:
                    rb_ps = h_pool.tile([128, CHUNK], F32, tag="h")
                    nc.tensor.matmul(
                        rb_ps, lhsT=ones_row, rhs=r_c, start=True, stop=True
                    )
                    rb_sb = r_pool.tile([128, CHUNK], F32, tag="rbc")
                    nc.scalar.activation(rb_sb, rb_ps, AF.Copy)
                    nc.vector.tensor_mul(cn_sb[:, nsl_c], ct_c, rb_sb)
                for g0 in range(0, N_CHUNKS, MLP_GROUP):
                    subs = range(g0, min(g0 + MLP_GROUP, N_CHUNKS))
                    net = {}
                    for sub in subs:
                        nsl = slice(sub * CHUNK, (sub + 1) * CHUNK)
                        for ht in range(2):
                            net_ps = n_pool.tile([128, CHUNK], F32, tag="net")
                            nc.tensor.matmul(
                                net_ps, lhsT=fcp_lhsT(ht), rhs=q4_sb[0:D, nsl],
                                start=True, stop=False,
                            )
                            nc.tensor.matmul(
                                net_ps, lhsT=fcc_lhsT(0, ht), rhs=cn_sb[:, nsl],
                                start=False, stop=True,
                            )
                            net[(ht, sub)] = net_ps

                    for i in range(NB):
                        for sub in subs:
                            nsl = slice(sub * CHUNK, (sub + 1) * CHUNK)
                            rx = []
                            for ht in range(2):
                                rx_sb = a_pool.tile([128, CHUNK], BF16, tag="rx")
                                nc.scalar.activation(
                                    rx_sb, net[(ht, sub)], AF.Relu,
                                    bias=bias_ap(ht, i),
                                )
                                rx.append(rx_sb)
                            h_tiles = []
                            for ht in range(2):
                                h_ps = h_pool.tile([128, CHUNK], F32, tag="h")
                                nc.tensor.matmul(
                                    h_ps, lhsT=b0_lhsT(i, 0, ht), rhs=rx[0],
                                    start=True, stop=False,
                                )
                                nc.tensor.matmul(
                                    h_ps, lhsT=b0_lhsT(i, 1, ht), rhs=rx[1],
                                    start=False, stop=True,
                                )
                                h_tiles.append(h_ps)
                            rh = []
                            for ht in range(2):
                                rh_sb = b_pool.tile([128, CHUNK], BF16, tag="rh")
                                nc.scalar.activation(
                                    rh_sb, h_tiles[ht], AF.Relu,
                                    bias=bias_ap(ht, 6 + i),
                                )
                                rh.append(rh_sb)
                            last = i == NB - 1
                            for ht in range(2):
                                nc.tensor.matmul(
                                    net[(ht, sub)], lhsT=b1_lhsT(i, 0, ht),
                                    rhs=rh[0], start=False, stop=False,
                                    skip_group_check=True,
                                )
                                nc.tensor.matmul(
                                    net[(ht, sub)], lhsT=b1_lhsT(i, 1, ht),
                                    rhs=rh[1], start=False, stop=last,
                                    skip_group_check=True,
                                )
                                if not last:
                                    nc.tensor.matmul(
                                        net[(ht, sub)], lhsT=fcc_lhsT(i + 1, ht),
                                        rhs=cn_sb[:, nsl], start=False, stop=True,
                                        skip_group_check=True,
                                    )

                    # out = out_W @ relu(net + B_y)   (+ out_b added on host)
                    for sub in subs:
                        nsl = slice(sub * CHUNK, (sub + 1) * CHUNK)
                        ry = []
                        for ht in range(2):
                            ry_sb = a_pool.tile([128, CHUNK], BF16, tag="rx")
                            nc.scalar.activation(
                                ry_sb, net[(ht, sub)], AF.Relu, bias=bias_ap(ht, 5)
                            )
                            ry.append(ry_sb)
                        o_ps = h_pool.tile([1, CHUNK], F32, tag="h")
                        nc.tensor.matmul(
                            o_ps, lhsT=ow_lhsT(0), rhs=ry[0], start=True, stop=False
                        )
                        nc.tensor.matmul(
                            o_ps, lhsT=ow_lhsT(1), rhs=ry[1], start=False, stop=True
                        )
                        out_sb = os_pool.tile([1, CHUNK], F32, tag="osb")
                        nc.scalar.activation(out_sb, o_ps, AF.Copy)
                        nc.sync.dma_start(out=out_d[:, nsl], in_=out_sb)

            r_pool_cm.__exit__(None, None, None)

    return nc


def host_prep(inputs):
    p = np.asarray(inputs["p"], np.float32)[0]      # [N, 3]
    pp = np.asarray(inputs["pp"], np.float32)[0]    # [M, 3]
    fea = np.ascontiguousarray(np.asarray(inputs["fea"], np.float32)[0])  # [M, C]

    qpack = np.empty((5, N), np.float32)
    qpack[0:3] = p.T
    qpack[3] = 1.0
    qpack[4] = (p * p).sum(1)
    ppack = np.empty((5, M), np.float32)
    ppack[0:3] = -2.0 * pp.T
    ppack[3] = (pp * pp).sum(1)
    ppack[4] = 1.0

    fc_p_W = np.asarray(inputs["fc_p_W"], np.float32)    # [H, 3]
    fc_c_W = np.asarray(inputs["fc_c_W"], np.float32)    # [NB, H, C]
    blk0_W = np.asarray(inputs["blk0_W"], np.float32)    # [NB, H, H]
    blk1_W = np.asarray(inputs["blk1_W"], np.float32)
    out_W = np.asarray(inputs["out_W"], np.float32)      # [1, H]
    fc_p_b = np.asarray(inputs["fc_p_b"], np.float32)
    fc_c_b = np.asarray(inputs["fc_c_b"], np.float32)    # [NB, H]
    blk0_b = np.asarray(inputs["blk0_b"], np.float32)
    blk1_b = np.asarray(inputs["blk1_b"], np.float32)
    out_b = float(np.asarray(inputs["out_b"], np.float32)[0])

    import ml_dtypes
    wtab = np.zeros((128, WCOLS), ml_dtypes.bfloat16)
    wtab[:, OFF_FCC : OFF_FCC + NB * H] = fc_c_W.transpose(2, 0, 1).reshape(C, NB * H)
    wtab[:, OFF_B0 : OFF_B0 + NB * 2 * H] = (
        blk0_W.reshape(NB, H, 2, 128).transpose(3, 0, 2, 1).reshape(128, NB * 2 * H)
    )
    wtab[:, OFF_B1 : OFF_B1 + NB * 2 * H] = (
        blk1_W.reshape(NB, H, 2, 128).transpose(3, 0, 2, 1).reshape(128, NB * 2 * H)
    )
    wtab[:, OFF_OW : OFF_OW + 2] = out_W.reshape(2, 128).T
    wtab[:, OFF_ONEC] = 1.0
    ftab = np.zeros((128, FCOLS), np.float32)

    # cumulative bias vectors folded into the Relu activations:
    #   vec 0..4  = B_i  (bias of net before block i's first relu)
    #   vec 5     = B_y  (bias of net before the final relu)
    #   vec 6..10 = blk0_b[i]  (bias of h before block i's second relu)
    vecs = np.zeros((11, H), np.float32)
    B = fc_p_b + fc_c_b[0]
    for i in range(NB):
        vecs[i] = B
        vecs[6 + i] = blk0_b[i]
        B = B + blk1_b[i] + (fc_c_b[i + 1] if i + 1 < NB else 0.0)
    vecs[5] = B
    ftab[:, OFF_BIAS : OFF_BIAS + 22] = (
        vecs.reshape(11, 2, 128).transpose(2, 1, 0).reshape(128, 22)
    )
    ftab[0, OFF_ONER : OFF_ONER + 128] = 1.0

    ftab[:, OFF_ONECF] = 1.0
    ftab[0:D, OFF_FCP : OFF_FCP + H] = fc_p_W.T
    # pp4: tile 4j+g lives at partitions 32g..32g+5, columns j*128..(j+1)*128
    pp4 = np.zeros((20, (M // 512) * 128), np.float32)
    for j in range(M // 512):
        for g in range(4):
            t = 4 * j + g
            pp4[5 * g : 5 * g + 5, j * 128 : (j + 1) * 128] = (
                ppack[:, t * 128 : (t + 1) * 128]
            )
    shared = {"fea": fea.astype(ml_dtypes.bfloat16), "wtab": wtab, "ftab": ftab,
              "pp4": pp4}
    in_maps = []
    for c in range(N_CORES):
        m = dict(shared)
        qc = qpack[:, c * NC_PER : (c + 1) * NC_PER]
        q4 = np.zeros((20, NC_PER), np.float32)
        for g in range(4):
            q4[5 * g : 5 * g + 5, :] = qc
        m["q4"] = q4
        in_maps.append(m)
    return in_maps, out_b


_NC_CACHE = {}


def kernel(**inputs) -> np.ndarray:
    in_maps, out_b = host_prep(inputs)
    if "nc" not in _NC_CACHE:
        nc = build_bass()
        nc.finalize()
        _NC_CACHE["nc"] = nc
    nc = _NC_CACHE["nc"]
    res = run_bass_kernel_spmd(nc, in_maps, list(range(N_CORES)))
    parts = [res.results[c]["out"] for c in range(N_CORES)]
    out = np.concatenate(parts, axis=1).astype(np.float32) + np.float32(out_b)
    return out


# revision 21
# speedup vs baseline: 2.7289x; 1.0054x over previous
"""Trainium2 Bass kernel for nn_LocalPointDecoder (sparse_attention).

Algorithm (per query point n):
  c[n]  = softmax_m(-|q_n - pp_m|^2 / VAR) @ fea          (Gaussian point attention)
  out[n] = MLP(c[n], q_n)                                  (5-block ResNet MLP, H=256)

Sharding: query points N=16384 split across 8 cores (2048 each); every core
holds the full pp/fea context and all MLP weights.

Device layout is fully transposed (features on partitions, queries on the
free axis):
  - d2^T [m, n] from ONE K=5 matmul of host-packed quadratic-form operands:
      ppack = [-2*pp_x, -2*pp_y, -2*pp_z, |pp|^2, 1]   (lhsT)
      qpack = [q_x, q_y, q_z, 1, |q|^2]                (rhs)
  - W^T = Exp(-d2/VAR) on ACT directly from PSUM (the reference's +EPS inside
    the square perturbs the softmax by <1e-4 relative — validated numerically).
  - c^T [C, n] and the softmax denominator accumulate over 32 m-tiles in PSUM
    (lhsT = fea tile / ones column).
  - MLP runs with H on partitions; weights host-pretransposed, layer biases
    folded into the Relu activations as cumulative per-partition bias vectors,
    the residual stream accumulates directly in PSUM (start=False matmuls).

Hardware constraint honored throughout: a Matmult carries at most ONE sem
wait (single LDWEIGHTS wait slot), so ACT is kept the last reader of every
PSUM bank and DMA waits are absorbed by observer instructions.
"""

import numpy as np

import concourse.bass as bass
import concourse.mybir as mybir
from concourse import bacc
import concourse.tile as tile
from concourse.bass_utils import run_bass_kernel_spmd

F32 = mybir.dt.float32
F32R = mybir.dt.float32r
BF16 = mybir.dt.bfloat16
AF = mybir.ActivationFunctionType

N_CORES = 8
N, M, D, C, H = 16384, 4096, 3, 128, 256
NB = 5
NC_PER = N // N_CORES          # 2048 queries per core
CHUNK = 512                    # free-dim tile (one fp32 PSUM bank)
N_CHUNKS = NC_PER // CHUNK     # 4
MT = M // 128                  # 32 context tiles
INV_VAR = 100.0                # 1 / (0.1**2)
MLP_GROUP = 2                  # sub-chunks resident in PSUM per MLP pass

# packed bf16 weight-table column offsets (one DMA for all matmul weights)
OFF_FCC = 0                      # [128, NB*H]   fc_c_W[i].T h-tiles
OFF_B0 = OFF_FCC + NB * H        # [128, NB*2*H] blk0_W[i].T (kt, ht)
OFF_B1 = OFF_B0 + NB * 2 * H
OFF_OW = OFF_B1 + NB * 2 * H     # [128, 2] out_W.T k-tiles
OFF_ONEC = OFF_OW + 2            # [128, 1] ones column (softmax denominator)
WCOLS = OFF_ONEC + 1
# f32 table: relu biases + fc_p weights + broadcast ones row
OFF_BIAS = 0                     # [128, 22] bias vecs (ht*11 + v)
OFF_FCP = OFF_BIAS + 22          # rows 0-2: fc_p_W.T h-tiles
OFF_ONER = OFF_FCP + H           # row 0: ones row (128 cols)
OFF_ONECF = OFF_ONER + 128       # [128, 1] f32 ones column (denom reduce)
FCOLS = OFF_ONECF + 1


def build_bass() -> bass.Bass:
    nc = bacc.Bacc()

    q4_d = nc.declare_dram_parameter("q4", [20, NC_PER], F32, isOutput=False)
    pp4_d = nc.declare_dram_parameter("pp4", [20, (MT // 4) * 128], F32, isOutput=False)
    fea_d = nc.declare_dram_parameter("fea", [M, C], BF16, isOutput=False)
    wtab_d = nc.declare_dram_parameter("wtab", [128, WCOLS], BF16, isOutput=False)
    ftab_d = nc.declare_dram_parameter("ftab", [128, FCOLS], F32, isOutput=False)
    out_d = nc.declare_dram_parameter("out", [1, NC_PER], F32, isOutput=True)
    import os
    dbg_d = None
    if os.environ.get("K_DEBUG"):
        dbg_d = nc.declare_dram_parameter("dbg", [C, NC_PER], BF16, isOutput=True)

    with tile.TileContext(nc) as tc:
        with tc.tile_pool(name="consts", bufs=1) as consts:
            # small, latency-critical inputs first so their DMA queues are
            # not serialized behind the big fea/wtab transfers
            q4_sb = consts.tile([128, NC_PER], F32, tag="q4")
            pp4_sb = consts.tile([128, (MT // 4) * 128], F32, tag="pp4")
            q20_sb = consts.tile([20, NC_PER], F32, tag="q20")
            nc.sync.dma_start(out=q20_sb, in_=q4_d[:, :])
            pp20_sb = consts.tile([20, (MT // 4) * 128], F32, tag="pp20")
            nc.sync.dma_start(out=pp20_sb, in_=pp4_d[:, :])
            for g in range(4):
                nc.sync.dma_start(
                    out=q4_sb[32 * g : 32 * g + 5, :],
                    in_=q20_sb[5 * g : 5 * g + 5, :],
                )
                nc.sync.dma_start(
                    out=pp4_sb[32 * g : 32 * g + 5, :],
                    in_=pp20_sb[5 * g : 5 * g + 5, :],
                )
            fea_sb = consts.tile([128, MT, C], BF16, tag="fea")
            fea_r = fea_d[:, :].rearrange("(t p) c -> p t c", p=128)
            for fq in range(4):
                nc.sync.dma_start(
                    out=fea_sb[:, fq * (MT // 4) : (fq + 1) * (MT // 4), :],
                    in_=fea_r[:, fq * (MT // 4) : (fq + 1) * (MT // 4), :],
                )
            wt_sb = consts.tile([128, WCOLS], BF16, tag="wtab")
            nc.sync.dma_start(out=wt_sb, in_=wtab_d[:, :])
            ft_sb = consts.tile([128, FCOLS], F32, tag="ftab")
            nc.sync.dma_start(out=ft_sb, in_=ftab_d[:, :])

            fcc_lhsT = lambda i, ht: wt_sb[
                :, OFF_FCC + i * H + ht * 128 : OFF_FCC + i * H + ht * 128 + 128
            ]
            b0_lhsT = lambda i, kt, ht: wt_sb[
                :,
                OFF_B0 + i * 512 + kt * 256 + ht * 128 : OFF_B0
                + i * 512 + kt * 256 + ht * 128 + 128,
            ]
            b1_lhsT = lambda i, kt, ht: wt_sb[
                :,
                OFF_B1 + i * 512 + kt * 256 + ht * 128 : OFF_B1
                + i * 512 + kt * 256 + ht * 128 + 128,
            ]
            fcp_lhsT = lambda ht: ft_sb[0:D, OFF_FCP + ht * 128 : OFF_FCP + ht * 128 + 128]
            ow_lhsT = lambda kt: wt_sb[:, OFF_OW + kt : OFF_OW + kt + 1]
            bias_ap = lambda ht, v: ft_sb[:, OFF_BIAS + ht * 11 + v : OFF_BIAS + ht * 11 + v + 1]
            ones_col = wt_sb[:, OFF_ONEC : OFF_ONEC + 1]
            ones_row = ft_sb[0:1, OFF_ONER : OFF_ONER + 128]
            ones_col_f = ft_sb[:, OFF_ONECF : OFF_ONECF + 1]

            cn_sb = consts.tile([C, NC_PER], BF16, tag="cn")  # normalized c^T

            # ---------------- attention phase ----------------
            r_pool_cm = tc.tile_pool(name="rsb", bufs=4)
            r_pool = r_pool_cm.__enter__()
            with (
                tc.tile_pool(name="spsum", bufs=5, space="PSUM") as s_pool,
                tc.tile_pool(name="ctpsum", bufs=2, space="PSUM") as c_pool,
                tc.tile_pool(name="dnpsum", bufs=1, space="PSUM") as d_pool,
                tc.tile_pool(name="wsb", bufs=6) as w_pool,
                tc.tile_pool(name="accsb", bufs=2) as a2_pool,
            ):
                NG = MT // 4  # 8 groups of 4 row-tiled d2 matmuls
                chunk_norm = []
                for ch in range(N_CHUNKS):
                    nsl = slice(ch * CHUNK, (ch + 1) * CHUNK)
                    ct_ps = c_pool.tile([C, CHUNK], F32, tag="ct")

                    def emit_s(j, nsl=nsl):
                        # 4 concurrent K=5 matmuls in distinct PE row groups
                        tiles = []
                        for g in range(4):
                            s_ps = s_pool.tile([128, CHUNK], F32, tag="s")
                            nc.tensor.matmul(
                                s_ps,
                                lhsT=pp4_sb[32 * g : 32 * g + 5,
                                            j * 128 : (j + 1) * 128],
                                rhs=q4_sb[32 * g : 32 * g + 5, nsl],
                                start=True,
                                stop=True,
                                tile_position=(32 * g, 0),
                            )
                            tiles.append(s_ps)
                        return tiles

                    s_tiles = {0: emit_s(0)}
                    if ch == 0:
                        # absorb the fea DMA wait into PE before the first
                        # c^T matmul (fea loads while the d2 matmuls run)
                        obs_ps = s_pool.tile([1, 1], F32, tag="s")
                        nc.tensor.matmul(
                            obs_ps, lhsT=fea_sb[0:1, 0, 0:1],
                            rhs=fea_sb[0:1, 0, 0:1], start=True, stop=True,
                        )
                    dn_acc = a2_pool.tile([128, CHUNK], F32, tag="dnacc")
                    for j in range(NG):
                        group = s_tiles.pop(j)
                        ws = []
                        for g in range(4):
                            w_sb = w_pool.tile([128, CHUNK], BF16, tag="w")
                            nc.scalar.activation(
                                w_sb, group[g], AF.Exp, scale=-INV_VAR
                            )
                            ws.append(w_sb)
                        if j + 1 < NG:
                            s_tiles[j + 1] = emit_s(j + 1)
                        for g in range(4):
                            t = 4 * j + g
                            nc.tensor.matmul(
                                ct_ps,
                                lhsT=fea_sb[:, t, :],
                                rhs=ws[g],
                                start=(t == 0),
                                stop=(t == MT - 1),
                            )
                            # denominator accumulates on the (idle) DVE
                            if t == 0:
                                nc.vector.tensor_copy(dn_acc, ws[g])
                            else:
                                nc.vector.tensor_add(dn_acc, dn_acc, ws[g])

                    # partition-reduce the DVE accumulator, then r = 1/denom,
                    # broadcast via a K=1 ones matmul, scale c^T.  Every PSUM
                    # tile's last reader stays ACT (single-wait rule).
                    dn_ps = d_pool.tile([1, CHUNK], F32, tag="dn")
                    nc.tensor.matmul(
                        dn_ps, lhsT=ones_col_f, rhs=dn_acc, start=True, stop=True
                    )
                    dn_sb = r_pool.tile([1, CHUNK], F32, tag="dnc")
                    nc.scalar.activation(dn_sb, dn_ps, AF.Copy)
                    r_sb = r_pool.tile([1, CHUNK], F32, tag="r")
                    nc.vector.reciprocal(r_sb, dn_sb)
                    ct_sb = r_pool.tile([C, CHUNK], F32, tag="ctc")
                    nc.scalar.activation(ct_sb, ct_ps, AF.Copy)
                    chunk_norm.append((nsl, r_sb, ct_sb))

            if dbg_d is not None:
                nc.sync.dma_start(out=dbg_d[:, :], in_=cn_sb)

            # ---------------- MLP phase ----------------
            # net^T stays resident in PSUM per (ht, sub); blk1/fc_c matmuls
            # accumulate the residual stream in place.  Two sub-chunks at a
            # time: 2 ht x 2 sub net banks + 3 h banks + obs = 8 banks.
            with (
                tc.tile_pool(name="netpsum", bufs=2 * MLP_GROUP, space="PSUM") as n_pool,
                tc.tile_pool(name="hpsum", bufs=3, space="PSUM") as h_pool,
                tc.tile_pool(name="asb", bufs=4) as a_pool,
                tc.tile_pool(name="bsb", bufs=4) as b_pool,
                tc.tile_pool(name="osb", bufs=2) as os_pool,
            ):
                for nsl_c, r_c, ct_c in chunk_norm:
                    rb_ps = h_pool.tile([128, CHUNK], F32, tag="h")
                    nc.tensor.matmul(
                        rb_ps, lhsT=ones_row, rhs=r_c, start=True, stop=True
                    )
                    rb_sb = r_pool.tile([128, CHUNK], F32, tag="rbc")
                    nc.scalar.activation(rb_sb, rb_ps, AF.Copy)
                    nc.vector.tensor_mul(cn_sb[:, nsl_c], ct_c, rb_sb)
                for g0 in range(0, N_CHUNKS, MLP_GROUP):
                    subs = range(g0, min(g0 + MLP_GROUP, N_CHUNKS))
                    net = {}
                    for sub in subs:
                        nsl = slice(sub * CHUNK, (sub + 1) * CHUNK)
                        for ht in range(2):
                            net_ps = n_pool.tile([128, CHUNK], F32, tag="net")
                            nc.tensor.matmul(
                                net_ps, lhsT=fcp_lhsT(ht), rhs=q4_sb[0:D, nsl],
                                start=True, stop=False,
                            )
                            nc.tensor.matmul(
                                net_ps, lhsT=fcc_lhsT(0, ht), rhs=cn_sb[:, nsl],
                                start=False, stop=True,
                            )
                            net[(ht, sub)] = net_ps

                    for i in range(NB):
                        for sub in subs:
                            nsl = slice(sub * CHUNK, (sub + 1) * CHUNK)
                            rx = []
                            for ht in range(2):
                                rx_sb = a_pool.tile([128, CHUNK], BF16, tag="rx")
                                nc.scalar.activation(
                                    rx_sb, net[(ht, sub)], AF.Relu,
                                    bias=bias_ap(ht, i),
                                )
                                rx.append(rx_sb)
                            h_tiles = []
                            for ht in range(2):
                                h_ps = h_pool.tile([128, CHUNK], F32, tag="h")
                                nc.tensor.matmul(
                                    h_ps, lhsT=b0_lhsT(i, 0, ht), rhs=rx[0],
                                    start=True, stop=False,
                                )
                                nc.tensor.matmul(
                                    h_ps, lhsT=b0_lhsT(i, 1, ht), rhs=rx[1],
                                    start=False, stop=True,
                                )
                                h_tiles.append(h_ps)
                            rh = []
                            for ht in range(2):
                                rh_sb = b_pool.tile([128, CHUNK], BF16, tag="rh")
                                nc.scalar.activation(
                                    rh_sb, h_tiles[ht], AF.Relu,
                                    bias=bias_ap(ht, 6 + i),
                                )
                                rh.append(rh_sb)
                            last = i == NB - 1
                            for ht in range(2):
                                nc.tensor.matmul(
                                    net[(ht, sub)], lhsT=b1_lhsT(i, 0, ht),
                                    rhs=rh[0], start=False, stop=False,
                                    skip_group_check=True,
                                )
                                nc.tensor.matmul(
                                    net[(ht, sub)], lhsT=b1_lhsT(i, 1, ht),
                                    rhs=rh[1], start=False, stop=last,
                                    skip_group_check=True,
                                )
                                if not last:
                                    nc.tensor.matmul(
                                        net[(ht, sub)], lhsT=fcc_lhsT(i + 1, ht),
                                        rhs=cn_sb[:, nsl], start=False, stop=True,
                                        skip_group_check=True,
                                    )

                    # out = out_W @ relu(net + B_y)   (+ out_b added on host)
                    for sub in subs:
                        nsl = slice(sub * CHUNK, (sub + 1) * CHUNK)
                        ry = []
                        for ht in range(2):
                            ry_sb = a_pool.tile([128, CHUNK], BF16, tag="rx")
                            nc.scalar.activation(
                                ry_sb, net[(ht, sub)], AF.Relu, bias=bias_ap(ht, 5)
                            )
                            ry.append(ry_sb)
                        o_ps = h_pool.tile([1, CHUNK], F32, tag="h")
                        nc.tensor.matmul(
                            o_ps, lhsT=ow_lhsT(0), rhs=ry[0], start=True, stop=False
                        )
                        nc.tensor.matmul(
                            o_ps, lhsT=ow_lhsT(1), rhs=ry[1], start=False, stop=True
                        )
                        out_sb = os_pool.tile([1, CHUNK], F32, tag="osb")
                        nc.scalar.activation(out_sb, o_ps, AF.Copy)
                        nc.sync.dma_start(out=out_d[:, nsl], in_=out_sb)

            r_pool_cm.__exit__(None, None, None)

    return nc


def host_prep(inputs):
    p = np.asarray(inputs["p"], np.float32)[0]      # [N, 3]
    pp = np.asarray(inputs["pp"], np.float32)[0]    # [M, 3]
    fea = np.ascontiguousarray(np.asarray(inputs["fea"], np.float32)[0])  # [M, C]

    qpack = np.empty((5, N), np.float32)
    qpack[0:3] = p.T
    qpack[3] = 1.0
    qpack[4] = (p * p).sum(1)
    ppack = np.empty((5, M), np.float32)
    ppack[0:3] = -2.0 * pp.T
    ppack[3] = (pp * pp).sum(1)
    ppack[4] = 1.0

    fc_p_W = np.asarray(inputs["fc_p_W"], np.float32)    # [H, 3]
    fc_c_W = np.asarray(inputs["fc_c_W"], np.float32)    # [NB, H, C]
    blk0_W = np.asarray(inputs["blk0_W"], np.float32)    # [NB, H, H]
    blk1_W = np.asarray(inputs["blk1_W"], np.float32)
    out_W = np.asarray(inputs["out_W"], np.float32)      # [1, H]
    fc_p_b = np.asarray(inputs["fc_p_b"], np.float32)
    fc_c_b = np.asarray(inputs["fc_c_b"], np.float32)    # [NB, H]
    blk0_b = np.asarray(inputs["blk0_b"], np.float32)
    blk1_b = np.asarray(inputs["blk1_b"], np.float32)
    out_b = float(np.asarray(inputs["out_b"], np.float32)[0])

    import ml_dtypes
    wtab = np.zeros((128, WCOLS), ml_dtypes.bfloat16)
    wtab[:, OFF_FCC : OFF_FCC + NB * H] = fc_c_W.transpose(2, 0, 1).reshape(C, NB * H)
    wtab[:, OFF_B0 : OFF_B0 + NB * 2 * H] = (
        blk0_W.reshape(NB, H, 2, 128).transpose(3, 0, 2, 1).reshape(128, NB * 2 * H)
    )
    wtab[:, OFF_B1 : OFF_B1 + NB * 2 * H] = (
        blk1_W.reshape(NB, H, 2, 128).transpose(3, 0, 2, 1).reshape(128, NB * 2 * H)
    )
    wtab[:, OFF_OW : OFF_OW + 2] = out_W.reshape(2, 128).T
    wtab[:, OFF_ONEC] = 1.0
    ftab = np.zeros((128, FCOLS), np.float32)

    # cumulative bias vectors folded into the Relu activations:
    #   vec 0..4  = B_i  (bias of net before block i's first relu)
    #   vec 5     = B_y  (bias of net before the final relu)
    #   vec 6..10 = blk0_b[i]  (bias of h before block i's second relu)
    vecs = np.zeros((11, H), np.float32)
    B = fc_p_b + fc_c_b[0]
    for i in range(NB):
        vecs[i] = B
        vecs[6 + i] = blk0_b[i]
        B = B + blk1_b[i] + (fc_c_b[i + 1] if i + 1 < NB else 0.0)
    vecs[5] = B
    ftab[:, OFF_BIAS : OFF_BIAS + 22] = (
        vecs.reshape(11, 2, 128).transpose(2, 1, 0).reshape(128, 22)
    )
    ftab[0, OFF_ONER : OFF_ONER + 128] = 1.0

    ftab[:, OFF_ONECF] = 1.0
    ftab[0:D, OFF_FCP : OFF_FCP + H] = fc_p_W.T
    # pp4: tile 4j+g lives at partitions 32g..32g+5, columns j*128..(j+1)*128
    pp4 = np.zeros((20, (M // 512) * 128), np.float32)
    for j in range(M // 512):
        for g in range(4):
            t = 4 * j + g
            pp4[5 * g : 5 * g + 5, j * 128 : (j + 1) * 128] = (
                ppack[:, t * 128 : (t + 1) * 128]
            )
    shared = {"fea": fea.astype(ml_dtypes.bfloat16), "wtab": wtab, "ftab": ftab,
              "pp4": pp4}
    in_maps = []
    for c in range(N_CORES):
        m = dict(shared)
        qc = qpack[:, c * NC_PER : (c + 1) * NC_PER]
        q4 = np.zeros((20, NC_PER), np.float32)
        for g in range(4):
            q4[5 * g : 5 * g + 5, :] = qc
        m["q4"] = q4
        in_maps.append(m)
    return in_maps, out_b


_NC_CACHE = {}


def kernel(**inputs) -> np.ndarray:
    in_maps, out_b = host_prep(inputs)
    if "nc" not in _NC_CACHE:
        nc = build_bass()
        nc.finalize()
        _NC_CACHE["nc"] = nc
    nc = _NC_CACHE["nc"]
    res = run_bass_kernel_spmd(nc, in_maps, list(range(N_CORES)))
    parts = [res.results[c]["out"] for c in range(N_CORES)]
    out = np.concatenate(parts, axis=1).astype(np.float32) + np.float32(out_b)
    return out
